# revision 32
# baseline (speedup 1.0000x reference)
"""Trainium2 kernel for nn_Attention_50182397886533.

Computation: LayerNorm + q/k + softmax on host (<3% of FLOPs); device
computes the dominant per-(query,key) value projection
    v[b,n,m,:] = xn[b,n,:] @ Wv[n,m]            (Wv: [65,65,128,256])
fused with the attention-weighted reduction over keys m.  Host applies
the small final Wout projection.

Sharding: query rows n across 8 cores — 8 full rows per core, plus the
straggler row 64 split over m (8 keys/core, core 7 gets 9).

Per-core device pipeline:
  - Wv streamed as bf16 in half-row granules (~17KB/partition) through a
    5-slot SBUF ring at the ~360 GB/s DMA roofline (~100us/core).
  - Rows processed in PAIRS: matmul chunk (2 keys x 256) for row A goes
    to PSUM partitions 0:64, row B to 64:128 of the same bank, so all
    post-matmul elementwise work runs at full 128-partition width.
  - attn * v multiply chunks are split between DVE and Pool engines
    (psum fp32 in, bf16 out to SBUF); the reduction over m is split by
    m-range between DVE and Pool; DVE adds the two partial sums.
  - ACT (scalar) engine issues output DMAs so the sync engine's granule
    prefetch stream is never blocked.
"""

import numpy as np

import concourse.bass as bass
import concourse.mybir as mybir
from concourse.bass_utils import run_bass_kernel_spmd

B = 64
N = 65
DIM = 128
HEADS = 8
DH = 32
INNER = 256
EPS = 1e-5

NPAIR = 4            # row pairs per core
NCHUNK = 33          # m-chunks per row (32 of width 2 + 1 of width 1)
ROW_COLS = N * INNER           # 16640 bf16 cols per row stream
NPCH = 5             # partial-row chunks (9 m slots: 4x2+1)
P_COLS = 9 * INNER             # partial-row stream cols (2304)

# chunk tables: (m0, mw) per j
CHUNKS = [(2 * j, 2) for j in range(32)] + [(64, 1)]
PCHUNKS = [(2 * j, 2) for j in range(4)] + [(8, 1)]

# Rows are streamed in QUARTERS; 8 granules per pair in consumption
# order A-q0, B-q0, A-q1, B-q1, ... Quarter q covers chunks JQ[q].
# weighted so the LAST quarter (the serial tail) is smallest
# DMA pieces per row (last ones small: they gate the serial drain tail)
JQ = [(0, 10), (10, 20), (20, 28), (28, 31), (31, 33)]
Q_OFF = [0, 5120, 10240, 14336, 15872]           # col offset of piece in row
Q_LEN = [5120, 5120, 4096, 1536, 768]            # cols per piece
NQ = len(JQ)
NSLOTS = 2 * NQ                # ring slots: one per piece-granule of a pair
# reduce pieces (independent of DMA pieces): chunk / m boundaries
RJ = [(0, 10), (10, 18), (18, 23), (23, 27), (27, 30), (30, 32), (32, 33)]
MR = [(0, 20), (20, 36), (36, 46), (46, 54), (54, 60), (60, 64), (64, 65)]
NR = len(RJ)                   # 7 reduce pieces -> 7 + 6 dself incs per pair

# Pool owns all multiply chunks; DVE owns the reductions.

_CACHED = {}


def _granule(g):
    """Granule g -> (dram col start, length). 2*NQ per pair: A/B per piece."""
    p, k = divmod(g, 2 * NQ)
    q, par = divmod(k, 2)
    base = p * 2 * ROW_COLS + par * ROW_COLS
    return base + Q_OFF[q], Q_LEN[q]


def _build_program():
    nc = bass.Bass()
    f32, bf16 = mybir.dt.float32, mybir.dt.bfloat16

    wv = nc.dram_tensor("wv", [DIM, 2 * NPAIR * ROW_COLS], bf16,
                        kind="ExternalInput")
    wvp = nc.dram_tensor("wvp", [DIM, P_COLS], bf16, kind="ExternalInput")
    xnT2 = nc.dram_tensor("xnT2", [DIM, NPAIR, 2 * B], bf16,
                          kind="ExternalInput")
    xnTp = nc.dram_tensor("xnTp", [DIM, B], bf16, kind="ExternalInput")
    attn2 = nc.dram_tensor("attn2", [2 * B, NPAIR, N, HEADS], bf16,
                           kind="ExternalInput")
    attnp = nc.dram_tensor("attnp", [B, 9, HEADS], bf16, kind="ExternalInput")
    out2 = nc.dram_tensor("out2", [2 * B, NPAIR, INNER], f32,
                          kind="ExternalOutput")
    outp = nc.dram_tensor("outp", [B, INNER], f32, kind="ExternalOutput")

    # build-time schedules -------------------------------------------------
    # global chunk order: partial-row chunks first (G 0..4), then pairs
    # (G = NPCH + NCHUNK*p + j).  Pool owns every multiply in this order.
    NG = NPCH + NPAIR * NCHUNK                # 137
    # matmul count after chunk G fully issued (pair chunks have 2 matmuls)
    mm_after = {}
    mm = 0
    for G in range(NG):
        mm += 1 if G < NPCH else 2
        mm_after[G] = mm
    # 4-chunk groups: partial row = groups 0,1; pair p group i (i<9) is
    # global group 2 + 9p + i (8 full groups of 4 chunks + chunk 32 alone).
    NGRP = 2 + 9 * NPAIR
    def grp_chunks(k):
        if k == 0:
            return list(range(4))
        if k == 1:
            return [4]
        p, i = divmod(k - 2, 9)
        j0 = 4 * i
        return [NPCH + NCHUNK * p + j for j in range(j0, min(j0 + 4, NCHUNK))]
    mm_after_grp = [mm_after[grp_chunks(k)[-1]] for k in range(NGRP)]
    acount = []
    _a = 0
    for k in range(NGRP):
        _a += 2 if len(grp_chunks(k)) == 4 else 1
        acount.append(_a)

    import contextlib
    with contextlib.ExitStack() as st:
        ring = [st.enter_context(nc.sbuf_tensor(f"ring{s}",
                                                [DIM, Q_LEN[s // 2]], bf16))
                for s in range(NSLOTS)]
        wvp_sb = st.enter_context(nc.sbuf_tensor([DIM, P_COLS], bf16))
        xnT2_sb = st.enter_context(nc.sbuf_tensor([DIM, NPAIR * 2 * B], bf16))
        xnTp_sb = st.enter_context(nc.sbuf_tensor([DIM, B], bf16))
        attn2_sb = st.enter_context(nc.sbuf_tensor([2 * B, NPAIR * N * HEADS],
                                                   bf16))
        attnp_sb = st.enter_context(nc.sbuf_tensor([B, 9 * HEADS], bf16))
        # PSUM: two 4-bank halves; groups alternate between them
        psA = st.enter_context(nc.psum_tensor([2 * B, 2048], f32))
        psB = st.enter_context(nc.psum_tensor([2 * B, 2048], f32))
        # ACT drain targets: 4-slot ring of raw v (bf16), partial-row vp
        vrow = [st.enter_context(nc.sbuf_tensor(f"vrow{s}", [2 * B, 2048],
                                                bf16)) for s in range(4)]
        vp = st.enter_context(nc.sbuf_tensor([B, P_COLS], bf16))
        # pool mult outputs: 2-slot scratch; L1 pair-sums; odd m=64 slice
        scr = [st.enter_context(nc.sbuf_tensor(f"scr{s}", [2 * B, 2048],
                                               bf16)) for s in range(2)]
        lvl1 = [st.enter_context(nc.sbuf_tensor(f"lvl1{s}", [2 * B, 8192],
                                                bf16)) for s in range(2)]
        osl = [st.enter_context(nc.sbuf_tensor(f"osl{s}", [2 * B, INNER],
                                               bf16)) for s in range(2)]
        sclp = st.enter_context(nc.sbuf_tensor([B, P_COLS], bf16))
        # DVE fold temps
        ta = [st.enter_context(nc.sbuf_tensor(f"ta{s}", [2 * B, 2048], bf16))
              for s in range(2)]
        ub = [[st.enter_context(nc.sbuf_tensor(f"ub{s}_{q}", [2 * B, 1024],
                                               bf16)) for q in range(2)]
              for s in range(2)]
        acc2 = [st.enter_context(nc.sbuf_tensor(f"acc2{s}", [2 * B, INNER],
                                                f32)) for s in range(2)]
        accp = st.enter_context(nc.sbuf_tensor([B, INNER], f32))

        hdrx = st.enter_context(nc.semaphore("hdrx"))
        hdra = st.enter_context(nc.semaphore("hdra"))
        gsem = [st.enter_context(nc.semaphore(f"gsem{s}"))
                for s in range(NSLOTS)]
        gpsem = st.enter_context(nc.semaphore("gpsem"))
        pe_mm = st.enter_context(nc.semaphore("pe_mm"))
        asem = st.enter_context(nc.semaphore("asem"))
        pmul = st.enter_context(nc.semaphore("pmul"))
        padd = st.enter_context(nc.semaphore("padd"))
        dself = st.enter_context(nc.semaphore("dself"))
        osem = st.enter_context(nc.semaphore("osem"))
        block = st.enter_context(nc.Block())

        # chunk -> (group, position) map
        chunk_grp = {}
        for k in range(NGRP):
            for pos, G in enumerate(grp_chunks(k)):
                chunk_grp[G] = (k, pos)
        ps = [psA, psB]

        # ---- sync engine: input DMA stream --------------------------------
        @block.sync
        def _(sy):
            sy.dma_start(xnT2_sb[:], xnT2.ap().rearrange("d p c -> d (p c)")
                         ).then_inc(hdrx, 16)
            sy.dma_start(xnTp_sb[:], xnTp.ap()).then_inc(hdrx, 16)
            for g in range(2 * NQ * NPAIR):
                if g == 1:
                    sy.dma_start(wvp_sb[:], wvp.ap()).then_inc(gpsem, 16)
                    # attention tensors: needed only once multiplies start
                    sy.dma_start(attn2_sb[:], attn2.ap().rearrange(
                        "c p m h -> c (p m h)")).then_inc(hdra, 16)
                    sy.dma_start(attnp_sb[:], attnp.ap().rearrange(
                        "c m h -> c (m h)")).then_inc(hdra, 16)
                if g >= NSLOTS:
                    # slot reuse: matmuls consuming the same piece of the
                    # previous pair must be done
                    gp, kp = divmod(g - NSLOTS, NSLOTS)
                    jlast = JQ[kp // 2][1] - 1
                    sy.wait_ge(pe_mm, mm_after[NPCH + NCHUNK * gp + jlast])
                c0, clen = _granule(g)
                sy.dma_start(ring[g % NSLOTS][:, :clen],
                             wv.ap()[:, c0:c0 + clen]).then_inc(
                                 gsem[g % NSLOTS], 16)

        # ---- tensor engine: value-projection matmuls ----------------------
        @block.tensor
        def _(t):
            t.wait_ge(hdrx, 32)         # xnT2 + xnTp loaded
            t.wait_ge(gpsem, 16)
            for j5 in range(NPCH):
                k, pos = chunk_grp[j5]
                m0, mw = PCHUNKS[j5]
                cols = mw * INNER
                t.matmul(ps[k % 2][0:B, 512 * pos:512 * pos + cols],
                         xnTp_sb[:], wvp_sb[:, 512 * j5:512 * j5 + cols],
                         start=True, stop=True).then_inc(pe_mm, 1)
            for p in range(NPAIR):
                for j in range(NCHUNK):
                    G = NPCH + NCHUNK * p + j
                    k, pos = chunk_grp[G]
                    q = next(i for i, (a, b) in enumerate(JQ) if a <= j < b)
                    ga, gb = NSLOTS * p + 2 * q, NSLOTS * p + 2 * q + 1
                    if pos == 0:
                        # psum half reuse: group k-2's drain copies done
                        t.wait_ge(asem, acount[k - 2])
                    m0, mw = CHUNKS[j]
                    cols = mw * INNER
                    off = 512 * (j - JQ[q][0])
                    bank = ps[k % 2]
                    if j == JQ[q][0]:
                        t.wait_ge(gsem[ga % NSLOTS], 16 * (ga // NSLOTS + 1))
                    t.matmul(bank[0:B, 512 * pos:512 * pos + cols],
                             xnT2_sb[:, p * 2 * B:p * 2 * B + B],
                             ring[ga % NSLOTS][:, off:off + cols],
                             start=True, stop=True).then_inc(pe_mm, 1)
                    if j == JQ[q][0]:
                        t.wait_ge(gsem[gb % NSLOTS], 16 * (gb // NSLOTS + 1))
                    t.matmul(bank[B:2 * B, 512 * pos:512 * pos + cols],
                             xnT2_sb[:, p * 2 * B + B:(p + 1) * 2 * B],
                             ring[gb % NSLOTS][:, off:off + cols],
                             start=True, stop=True).then_inc(pe_mm, 1)

        # ---- ACT: PSUM -> SBUF drain copies (bf16) + output DMAs ----------
        @block.scalar
        def _(s):
            for k in range(NGRP):
                if k >= 2 and len(grp_chunks(k)) == 4:
                    # vrow slot reuse: pool mult of group k-4 done
                    if k >= 6:
                        s.wait_ge(pmul, k - 3)
                    dst = vp if k == 0 else vrow[k % 4]
                    # two half-copies so the drain interleaves finer
                    s.wait_ge(pe_mm, mm_after[grp_chunks(k)[1]])
                    s.copy(dst[:, :1024],
                           ps[k % 2][:, :1024]).then_inc(asem, 1)
                    s.wait_ge(pe_mm, mm_after_grp[k])
                    s.copy(dst[:, 1024:2048],
                           ps[k % 2][:, 1024:2048]).then_inc(asem, 1)
                elif k == 0:
                    s.wait_ge(pe_mm, mm_after[grp_chunks(0)[1]])
                    s.copy(vp[:, :1024], ps[0][0:B, :1024]).then_inc(asem, 1)
                    s.wait_ge(pe_mm, mm_after_grp[0])
                    s.copy(vp[:, 1024:2048],
                           ps[0][0:B, 1024:2048]).then_inc(asem, 1)
                elif k == 1:
                    s.wait_ge(pe_mm, mm_after_grp[1])
                    s.copy(vp[:, 2048:], ps[1][0:B, :256]).then_inc(asem, 1)
                else:
                    p, i = divmod(k - 2, 9)
                    s.wait_ge(pe_mm, mm_after_grp[k])
                    if k >= 6:
                        s.wait_ge(pmul, k - 3)
                    s.copy(vrow[k % 4][:, :256],
                           ps[k % 2][:, :256]).then_inc(asem, 1)
                if k == 2:
                    s.wait_ge(dself, 1)
                    s.dma_start(outp.ap()[:], accp[:]).then_inc(osem, 16)
                if k >= 11 and (k - 2) % 9 == 0:
                    # pair (k-2)//9 - 1 finished: ship it
                    pq = (k - 2) // 9 - 1
                    s.wait_ge(dself, 8 * pq + 9)
                    s.dma_start(out2.ap()[:, pq, :],
                                acc2[pq % 2][:]).then_inc(osem, 16)
            s.wait_ge(dself, 8 * (NPAIR - 1) + 9)
            s.dma_start(out2.ap()[:, NPAIR - 1, :],
                        acc2[(NPAIR - 1) % 2][:]).then_inc(osem, 16)

        # ---- Pool: attention multiplies + first-level pair sums -----------
        @block.gpsimd
        def _(g_):
            g_.wait_ge(hdra, 32)
            attn4 = attn2_sb[:].rearrange("q (p m h) -> q p m h",
                                          p=NPAIR, m=N)
            attnp4 = attnp_sb[:].rearrange("q (m h) -> q m h", m=9)
            for k in range(NGRP):
                g_.wait_ge(asem, acount[k])
                if k == 0:
                    g_.tensor_tensor(
                        sclp[:, :2048].rearrange("q (m h d) -> q m h d",
                                                 m=8, h=HEADS),
                        vp[:, :2048].rearrange("q (m h d) -> q m h d",
                                               m=8, h=HEADS),
                        attnp4[:, 0:8, :, None].to_broadcast(
                            (B, 8, HEADS, DH)),
                        mybir.AluOpType.mult).then_inc(pmul, 1)
                    continue
                if k == 1:
                    g_.tensor_tensor(
                        sclp[:, 2048:].rearrange("q (m h d) -> q m h d",
                                                 m=1, h=HEADS),
                        vp[:, 2048:].rearrange("q (m h d) -> q m h d",
                                               m=1, h=HEADS),
                        attnp4[:, 8:9, :, None].to_broadcast(
                            (B, 1, HEADS, DH)),
                        mybir.AluOpType.mult).then_inc(pmul, 1)
                    continue
                p, i = divmod(k - 2, 9)
                m0 = 8 * i
                mw = 8 if i < 8 else 1
                if i == 0 and p >= 2:
                    # lvl1/osl[p%2] reuse: DVE finished pair p-2
                    g_.wait_ge(dself, 8 * (p - 2) + 9)
                if i < 8:
                    if 8 * p + i >= 2:
                        g_.wait_ge(padd, 8 * p + i - 1)   # scr[i%2] free
                    out_ap = scr[i % 2][:].rearrange(
                        "q (m h d) -> q m h d", m=mw, h=HEADS)
                else:
                    out_ap = osl[p % 2][:].rearrange(
                        "q (m h d) -> q m h d", m=mw, h=HEADS)
                g_.tensor_tensor(
                    out_ap,
                    vrow[k % 4][:, :mw * INNER].rearrange(
                        "q (m h d) -> q m h d", m=mw, h=HEADS),
                    attn4[:, p, m0:m0 + mw, :, None].to_broadcast(
                        (2 * B, mw, HEADS, DH)),
                    mybir.AluOpType.mult).then_inc(pmul, 1)
                if i < 8:
                    g_.wait_ge(pmul, k + 1)           # own mult retired
                    g_.tensor_tensor(
                        lvl1[p % 2][:, 1024 * i:1024 * (i + 1)],
                        scr[i % 2][:, :1024], scr[i % 2][:, 1024:],
                        mybir.AluOpType.add).then_inc(padd, 1)

        # ---- DVE: partial reduce + bf16 fold tree per pair ----------------
        # dself incs: 1 (partial) + 8 per pair.
        @block.vector
        def _(v):
            v.wait_ge(pmul, 2)
            v.tensor_reduce(
                accp[:].rearrange("q (h d) -> q h d", h=HEADS),
                sclp[:].rearrange("q (m h d) -> q h d m", m=9, h=HEADS),
                axis=mybir.AxisListType.X,
                op=mybir.AluOpType.add).then_inc(dself, 1)
            for p in range(NPAIR):
                base = 1 + 8 * p
                pl = p % 2
                if p >= 2:
                    # all out DMAs issued so far (outp + pairs 0..p-1) done
                    v.wait_ge(osem, 16 * (p + 1))
                v.wait_ge(padd, 8 * p + 4)            # slabs 0..3 summed
                v.wait_ge(dself, base)                # ta/u free (pair p-1)
                v.tensor_tensor(ta[pl][:], lvl1[pl][:, :2048],
                                lvl1[pl][:, 2048:4096],
                                mybir.AluOpType.add).then_inc(dself, 1)
                v.wait_ge(dself, base + 1)
                v.tensor_tensor(ub[pl][0][:], ta[pl][:, :1024],
                                ta[pl][:, 1024:],
                                mybir.AluOpType.add).then_inc(dself, 1)
                v.wait_ge(padd, 8 * p + 8)            # slabs 4..7 summed
                v.wait_ge(dself, base + 2)
                v.tensor_tensor(ta[pl][:], lvl1[pl][:, 4096:6144],
                                lvl1[pl][:, 6144:8192],
                                mybir.AluOpType.add).then_inc(dself, 1)
                v.wait_ge(dself, base + 3)
                v.tensor_tensor(ub[pl][1][:], ta[pl][:, :1024],
                                ta[pl][:, 1024:],
                                mybir.AluOpType.add).then_inc(dself, 1)
                v.wait_ge(dself, base + 4)
                v.tensor_tensor(ta[pl][:, :1024], ub[pl][0][:], ub[pl][1][:],
                                mybir.AluOpType.add).then_inc(dself, 1)
                v.wait_ge(dself, base + 5)
                v.tensor_tensor(ub[pl][0][:, :512], ta[pl][:, :512],
                                ta[pl][:, 512:1024],
                                mybir.AluOpType.add).then_inc(dself, 1)
                v.wait_ge(dself, base + 6)
                v.tensor_tensor(ta[pl][:, :256], ub[pl][0][:, :256],
                                ub[pl][0][:, 256:512],
                                mybir.AluOpType.add).then_inc(dself, 1)
                v.wait_ge(pmul, 2 + 9 * p + 9)        # odd-slice mult done
                v.wait_ge(dself, base + 7)
                v.tensor_tensor(acc2[pl][:], ta[pl][:, :256], osl[pl][:],
                                mybir.AluOpType.add).then_inc(dself, 1)

    return nc


def _to_bf16(a):
    import ml_dtypes
    return np.asarray(a, dtype=ml_dtypes.bfloat16)


def host_prep(x, gamma, beta, Wqk, Wv, Wout, bout):
    """LayerNorm + qk + softmax on host; build per-core in_maps."""
    x = np.asarray(x, np.float32)
    mu = x.mean(-1, keepdims=True)
    var = np.square(x - mu).mean(-1, keepdims=True)
    xn = ((x - mu) / np.sqrt(var + EPS) * np.asarray(gamma, np.float32)
          + np.asarray(beta, np.float32)).astype(np.float32)

    qk = xn @ np.asarray(Wqk, np.float32)
    q, k = qk[..., :INNER], qk[..., INNER:]
    q = q.reshape(B, N, HEADS, DH).transpose(0, 2, 1, 3)
    k = k.reshape(B, N, HEADS, DH).transpose(0, 2, 1, 3)
    dots = np.einsum("bhnd,bhmd->bhnm", q, k) * (DH ** -0.5)
    dots -= dots.max(-1, keepdims=True)
    e = np.exp(dots)
    attn = (e / e.sum(-1, keepdims=True)).astype(np.float32)  # [b,h,n,m]

    # [n, d, m*e] bf16 weight stream source
    WvT = np.ascontiguousarray(
        _to_bf16(Wv).transpose(0, 2, 1, 3).reshape(N, DIM, N * INNER))
    xnT = _to_bf16(xn.transpose(2, 1, 0))       # [d, n, b]

    in_maps = []
    for c in range(8):
        rows = [8 * c + i for i in range(8)]
        # wv stream: rows in natural order [A0|B0|A1|B1|...]; quarter
        # granules address strided slices of this layout directly.
        wv_g = np.empty((DIM, 2 * NPAIR * ROW_COLS), WvT.dtype)
        for p in range(NPAIR):
            wv_g[:, (2 * p) * ROW_COLS:(2 * p + 1) * ROW_COLS] = \
                WvT[rows[2 * p]]
            wv_g[:, (2 * p + 1) * ROW_COLS:(2 * p + 2) * ROW_COLS] = \
                WvT[rows[2 * p + 1]]
        # partial row m-range
        mstart, mcount = 8 * c, (9 if c == 7 else 8)
        wvp_c = np.zeros((DIM, P_COLS), WvT.dtype)
        wvp_c[:, :mcount * INNER] = WvT[64][
            :, mstart * INNER:(mstart + mcount) * INNER]
        # activations / attention
        xnT2_c = np.empty((DIM, NPAIR, 2 * B), xnT.dtype)
        attn2_c = np.empty((2 * B, NPAIR, N, HEADS), np.float32)
        for p in range(NPAIR):
            for par in range(2):
                nrow = rows[2 * p + par]
                xnT2_c[:, p, par * B:(par + 1) * B] = xnT[:, nrow, :]
                attn2_c[par * B:(par + 1) * B, p] = \
                    attn[:, :, nrow, :].transpose(0, 2, 1)
        attnp_c = np.zeros((B, 9, HEADS), np.float32)
        attnp_c[:, :mcount] = attn[
            :, :, 64, mstart:mstart + mcount].transpose(0, 2, 1)
        in_maps.append({"wv": wv_g, "wvp": wvp_c, "xnT2": xnT2_c,
                        "xnTp": _to_bf16(xn[:, 64, :].T),
                        "attn2": _to_bf16(attn2_c),
                        "attnp": _to_bf16(attnp_c)})
    return in_maps, xn, attn


def assemble(results, Wout, bout):
    out_pre = np.zeros((B, N, INNER), np.float32)
    for c in range(8):
        o2 = np.asarray(results[c]["out2"], np.float32)  # [2B, NPAIR, INNER]
        for p in range(NPAIR):
            out_pre[:, 8 * c + 2 * p, :] = o2[:B, p]
            out_pre[:, 8 * c + 2 * p + 1, :] = o2[B:, p]
        out_pre[:, 64, :] += np.asarray(results[c]["outp"], np.float32)
    out = out_pre.reshape(B * N, INNER) @ np.asarray(Wout, np.float32) \
        + np.asarray(bout, np.float32)
    return out.reshape(B, N, DIM).astype(np.float32)


def kernel(x, gamma, beta, Wqk, Wv, Wout, bout):
    in_maps, _, _ = host_prep(x, gamma, beta, Wqk, Wv, Wout, bout)
    if "nc" not in _CACHED:
        _CACHED["nc"] = _build_program()
    res = run_bass_kernel_spmd(_CACHED["nc"], in_maps, list(range(8))).results
    return assemble(res, Wout, bout)


# revision 35
# speedup vs baseline: 1.0193x; 1.0193x over previous
"""Trainium2 kernel for nn_Attention_50182397886533.

Computation: LayerNorm + q/k + softmax on host (<3% of FLOPs); device
computes the dominant per-(query,key) value projection
    v[b,n,m,:] = xn[b,n,:] @ Wv[n,m]            (Wv: [65,65,128,256])
fused with the attention-weighted reduction over keys m.  Host applies
the small final Wout projection.

Sharding: query rows n across 8 cores — 8 full rows per core, plus the
straggler row 64 split over m (8 keys/core, core 7 gets 9).

Per-core device pipeline (DMA-bound at ~360 GB/s, ~108us stream):
  - Wv streamed as bf16 row-piece granules through a 10-slot SBUF ring
    issued by the sync engine (SP); piece sizes shrink toward the end of
    each row so the serial drain tail stays short.
  - Rows processed in PAIRS: each matmul chunk (2 keys x 256) for row A
    goes to PSUM partitions 0:64, row B to 64:128 of the same 4-bank
    half, so all post-matmul work runs at full 128-partition width.
  - ACT (scalar) drains PSUM -> SBUF as bf16 1024-wide copies (GPSIMD
    cannot access PSUM; DVE reads of PSUM get no fast modes).
  - Pool (gpsimd) applies the attention weights: one 8-key-wide
    tensor_tensor multiply per drained group, SBUF-only.
  - DVE sums over keys with a bf16 binary fold tree (packed bf16 adds
    get the DVE 2x mode; tensor_reduce would run at 1x), interleaved
    with pool progress, then ships acc via ACT-issued output DMAs.
"""

import numpy as np

import concourse.bass as bass
import concourse.mybir as mybir
from concourse.bass_utils import run_bass_kernel_spmd

B = 64
N = 65
DIM = 128
HEADS = 8
DH = 32
INNER = 256
EPS = 1e-5

NPAIR = 4            # row pairs per core
NCHUNK = 33          # m-chunks per row (32 of width 2 + 1 of width 1)
ROW_COLS = N * INNER           # 16640 bf16 cols per row stream
NPCH = 5             # partial-row chunks (9 m slots: 4x2+1)
P_COLS = 9 * INNER             # partial-row stream cols (2304)

# chunk tables: (m0, mw) per j
CHUNKS = [(2 * j, 2) for j in range(32)] + [(64, 1)]
PCHUNKS = [(2 * j, 2) for j in range(4)] + [(8, 1)]

# Rows are streamed in QUARTERS; 8 granules per pair in consumption
# order A-q0, B-q0, A-q1, B-q1, ... Quarter q covers chunks JQ[q].
# weighted so the LAST quarter (the serial tail) is smallest
# DMA pieces per row (last ones small: they gate the serial drain tail)
JQ = [(0, 10), (10, 20), (20, 28), (28, 31), (31, 33)]
Q_OFF = [0, 5120, 10240, 14336, 15872]           # col offset of piece in row
Q_LEN = [5120, 5120, 4096, 1536, 768]            # cols per piece
NQ = len(JQ)
NSLOTS = 2 * NQ                # ring slots: one per piece-granule of a pair


# Pool owns all multiply chunks; DVE owns the reductions.

_CACHED = {}


def _granule(g):
    """Granule g -> (dram col start, length). 2*NQ per pair: A/B per piece."""
    p, k = divmod(g, 2 * NQ)
    q, par = divmod(k, 2)
    base = p * 2 * ROW_COLS + par * ROW_COLS
    return base + Q_OFF[q], Q_LEN[q]


def _build_program():
    nc = bass.Bass()
    f32, bf16 = mybir.dt.float32, mybir.dt.bfloat16

    wv = nc.dram_tensor("wv", [DIM, 2 * NPAIR * ROW_COLS], bf16,
                        kind="ExternalInput")
    wvp = nc.dram_tensor("wvp", [DIM, P_COLS], bf16, kind="ExternalInput")
    xnT2 = nc.dram_tensor("xnT2", [DIM, NPAIR, 2 * B], bf16,
                          kind="ExternalInput")
    xnTp = nc.dram_tensor("xnTp", [DIM, B], bf16, kind="ExternalInput")
    attn2 = nc.dram_tensor("attn2", [2 * B, NPAIR, N, HEADS], bf16,
                           kind="ExternalInput")
    attnp = nc.dram_tensor("attnp", [B, 9, HEADS], bf16, kind="ExternalInput")
    out2 = nc.dram_tensor("out2", [2 * B, NPAIR, INNER], f32,
                          kind="ExternalOutput")
    outp = nc.dram_tensor("outp", [B, INNER], f32, kind="ExternalOutput")

    # build-time schedules -------------------------------------------------
    # global chunk order: partial-row chunks first (G 0..4), then pairs
    # (G = NPCH + NCHUNK*p + j).  Pool owns every multiply in this order.
    NG = NPCH + NPAIR * NCHUNK                # 137
    # matmul count after chunk G fully issued (pair chunks have 2 matmuls)
    mm_after = {}
    mm = 0
    for G in range(NG):
        mm += 1 if G < NPCH else 2
        mm_after[G] = mm
    # 4-chunk groups: partial row = groups 0,1; pair p group i (i<9) is
    # global group 2 + 9p + i (8 full groups of 4 chunks + chunk 32 alone).
    NGRP = 2 + 9 * NPAIR
    def grp_chunks(k):
        if k == 0:
            return list(range(4))
        if k == 1:
            return [4]
        p, i = divmod(k - 2, 9)
        j0 = 4 * i
        return [NPCH + NCHUNK * p + j for j in range(j0, min(j0 + 4, NCHUNK))]
    mm_after_grp = [mm_after[grp_chunks(k)[-1]] for k in range(NGRP)]
    acount = []
    _a = 0
    for k in range(NGRP):
        _a += 2 if len(grp_chunks(k)) == 4 else 1
        acount.append(_a)

    import contextlib
    with contextlib.ExitStack() as st:
        ring = [st.enter_context(nc.sbuf_tensor(f"ring{s}",
                                                [DIM, Q_LEN[s // 2]], bf16))
                for s in range(NSLOTS)]
        wvp_sb = st.enter_context(nc.sbuf_tensor([DIM, P_COLS], bf16))
        xnT2_sb = st.enter_context(nc.sbuf_tensor([DIM, NPAIR * 2 * B], bf16))
        xnTp_sb = st.enter_context(nc.sbuf_tensor([DIM, B], bf16))
        attn2_sb = st.enter_context(nc.sbuf_tensor([2 * B, NPAIR * N * HEADS],
                                                   bf16))
        attnp_sb = st.enter_context(nc.sbuf_tensor([B, 9 * HEADS], bf16))
        # PSUM: two 4-bank halves; groups alternate between them
        psA = st.enter_context(nc.psum_tensor([2 * B, 2048], f32))
        psB = st.enter_context(nc.psum_tensor([2 * B, 2048], f32))
        # ACT drain targets: 4-slot ring of raw v (bf16), partial-row vp
        vrow = [st.enter_context(nc.sbuf_tensor(f"vrow{s}", [2 * B, 2048],
                                                bf16)) for s in range(4)]
        vp = st.enter_context(nc.sbuf_tensor([B, P_COLS], bf16))
        # pool mult outputs: 2-slot scratch; L1 pair-sums; odd m=64 slice
        scr = [st.enter_context(nc.sbuf_tensor(f"scr{s}", [2 * B, 2048],
                                               bf16)) for s in range(2)]
        lvl1 = [st.enter_context(nc.sbuf_tensor(f"lvl1{s}", [2 * B, 8192],
                                                bf16)) for s in range(2)]
        osl = [st.enter_context(nc.sbuf_tensor(f"osl{s}", [2 * B, INNER],
                                               bf16)) for s in range(2)]
        sclp = st.enter_context(nc.sbuf_tensor([B, P_COLS], bf16))
        # DVE fold temps
        ta = [st.enter_context(nc.sbuf_tensor(f"ta{s}", [2 * B, 2048], bf16))
              for s in range(2)]
        ub = [[st.enter_context(nc.sbuf_tensor(f"ub{s}_{q}", [2 * B, 1024],
                                               bf16)) for q in range(2)]
              for s in range(2)]
        acc2 = [st.enter_context(nc.sbuf_tensor(f"acc2{s}", [2 * B, INNER],
                                                f32)) for s in range(2)]
        accp = st.enter_context(nc.sbuf_tensor([B, INNER], f32))

        hdrx = st.enter_context(nc.semaphore("hdrx"))
        hdra = st.enter_context(nc.semaphore("hdra"))
        gsem = [st.enter_context(nc.semaphore(f"gsem{s}"))
                for s in range(NSLOTS)]
        gpsem = st.enter_context(nc.semaphore("gpsem"))
        pe_mm = st.enter_context(nc.semaphore("pe_mm"))
        asem = st.enter_context(nc.semaphore("asem"))
        pmul = st.enter_context(nc.semaphore("pmul"))
        padd = st.enter_context(nc.semaphore("padd"))
        dself = st.enter_context(nc.semaphore("dself"))
        osem = st.enter_context(nc.semaphore("osem"))
        block = st.enter_context(nc.Block())

        # chunk -> (group, position) map
        chunk_grp = {}
        for k in range(NGRP):
            for pos, G in enumerate(grp_chunks(k)):
                chunk_grp[G] = (k, pos)
        ps = [psA, psB]

        # ---- sync engine: input DMA stream --------------------------------
        @block.sync
        def _(sy):
            sy.dma_start(xnT2_sb[:], xnT2.ap().rearrange("d p c -> d (p c)")
                         ).then_inc(hdrx, 16)
            sy.dma_start(xnTp_sb[:], xnTp.ap()).then_inc(hdrx, 16)
            for g in range(2 * NQ * NPAIR):
                if g == 1:
                    sy.dma_start(wvp_sb[:], wvp.ap()).then_inc(gpsem, 16)
                    # attention tensors: needed only once multiplies start
                    sy.dma_start(attn2_sb[:], attn2.ap().rearrange(
                        "c p m h -> c (p m h)")).then_inc(hdra, 16)
                    sy.dma_start(attnp_sb[:], attnp.ap().rearrange(
                        "c m h -> c (m h)")).then_inc(hdra, 16)
                if g >= NSLOTS:
                    # slot reuse: matmuls consuming the same piece of the
                    # previous pair must be done
                    gp, kp = divmod(g - NSLOTS, NSLOTS)
                    jlast = JQ[kp // 2][1] - 1
                    sy.wait_ge(pe_mm, mm_after[NPCH + NCHUNK * gp + jlast])
                c0, clen = _granule(g)
                sy.dma_start(ring[g % NSLOTS][:, :clen],
                             wv.ap()[:, c0:c0 + clen]).then_inc(
                                 gsem[g % NSLOTS], 16)

        # ---- tensor engine: value-projection matmuls ----------------------
        @block.tensor
        def _(t):
            t.wait_ge(hdrx, 32)         # xnT2 + xnTp loaded
            t.wait_ge(gpsem, 16)
            for j5 in range(NPCH):
                k, pos = chunk_grp[j5]
                m0, mw = PCHUNKS[j5]
                cols = mw * INNER
                t.matmul(ps[k % 2][0:B, 512 * pos:512 * pos + cols],
                         xnTp_sb[:], wvp_sb[:, 512 * j5:512 * j5 + cols],
                         start=True, stop=True).then_inc(pe_mm, 1)
            for p in range(NPAIR):
                for j in range(NCHUNK):
                    G = NPCH + NCHUNK * p + j
                    k, pos = chunk_grp[G]
                    q = next(i for i, (a, b) in enumerate(JQ) if a <= j < b)
                    ga, gb = NSLOTS * p + 2 * q, NSLOTS * p + 2 * q + 1
                    if pos == 0:
                        # psum half reuse: group k-2's drain copies done
                        t.wait_ge(asem, acount[k - 2])
                    m0, mw = CHUNKS[j]
                    cols = mw * INNER
                    off = 512 * (j - JQ[q][0])
                    bank = ps[k % 2]
                    if j == JQ[q][0]:
                        t.wait_ge(gsem[ga % NSLOTS], 16 * (ga // NSLOTS + 1))
                    t.matmul(bank[0:B, 512 * pos:512 * pos + cols],
                             xnT2_sb[:, p * 2 * B:p * 2 * B + B],
                             ring[ga % NSLOTS][:, off:off + cols],
                             start=True, stop=True).then_inc(pe_mm, 1)
                    if j == JQ[q][0]:
                        t.wait_ge(gsem[gb % NSLOTS], 16 * (gb // NSLOTS + 1))
                    t.matmul(bank[B:2 * B, 512 * pos:512 * pos + cols],
                             xnT2_sb[:, p * 2 * B + B:(p + 1) * 2 * B],
                             ring[gb % NSLOTS][:, off:off + cols],
                             start=True, stop=True).then_inc(pe_mm, 1)

        # ---- ACT: PSUM -> SBUF drain copies (bf16) + output DMAs ----------
        @block.scalar
        def _(s):
            for k in range(NGRP):
                if k >= 2 and len(grp_chunks(k)) == 4:
                    # vrow slot reuse: pool mult of group k-4 done
                    if k >= 6:
                        s.wait_ge(pmul, k - 3)
                    dst = vp if k == 0 else vrow[k % 4]
                    # two half-copies so the drain interleaves finer
                    s.wait_ge(pe_mm, mm_after[grp_chunks(k)[1]])
                    s.copy(dst[:, :1024],
                           ps[k % 2][:, :1024]).then_inc(asem, 1)
                    s.wait_ge(pe_mm, mm_after_grp[k])
                    s.copy(dst[:, 1024:2048],
                           ps[k % 2][:, 1024:2048]).then_inc(asem, 1)
                elif k == 0:
                    s.wait_ge(pe_mm, mm_after[grp_chunks(0)[1]])
                    s.copy(vp[:, :1024], ps[0][0:B, :1024]).then_inc(asem, 1)
                    s.wait_ge(pe_mm, mm_after_grp[0])
                    s.copy(vp[:, 1024:2048],
                           ps[0][0:B, 1024:2048]).then_inc(asem, 1)
                elif k == 1:
                    s.wait_ge(pe_mm, mm_after_grp[1])
                    s.copy(vp[:, 2048:], ps[1][0:B, :256]).then_inc(asem, 1)
                else:
                    p, i = divmod(k - 2, 9)
                    s.wait_ge(pe_mm, mm_after_grp[k])
                    if k >= 6:
                        s.wait_ge(pmul, k - 3)
                    s.copy(vrow[k % 4][:, :256],
                           ps[k % 2][:, :256]).then_inc(asem, 1)
                if k == 2:
                    s.wait_ge(dself, 1)
                    s.dma_start(outp.ap()[:], accp[:]).then_inc(osem, 16)
                if k >= 11 and (k - 2) % 9 == 0:
                    # pair (k-2)//9 - 1 finished: ship it
                    pq = (k - 2) // 9 - 1
                    s.wait_ge(dself, 8 * pq + 9)
                    s.dma_start(out2.ap()[:, pq, :],
                                acc2[pq % 2][:]).then_inc(osem, 16)
            s.wait_ge(dself, 8 * (NPAIR - 1) + 9)
            s.dma_start(out2.ap()[:, NPAIR - 1, :],
                        acc2[(NPAIR - 1) % 2][:]).then_inc(osem, 16)

        # ---- Pool: attention multiplies + first-level pair sums -----------
        @block.gpsimd
        def _(g_):
            g_.wait_ge(hdra, 32)
            attn4 = attn2_sb[:].rearrange("q (p m h) -> q p m h",
                                          p=NPAIR, m=N)
            attnp4 = attnp_sb[:].rearrange("q (m h) -> q m h", m=9)
            for k in range(NGRP):
                g_.wait_ge(asem, acount[k])
                if k == 0:
                    g_.tensor_tensor(
                        sclp[:, :2048].rearrange("q (m h d) -> q m h d",
                                                 m=8, h=HEADS),
                        vp[:, :2048].rearrange("q (m h d) -> q m h d",
                                               m=8, h=HEADS),
                        attnp4[:, 0:8, :, None].to_broadcast(
                            (B, 8, HEADS, DH)),
                        mybir.AluOpType.mult).then_inc(pmul, 1)
                    continue
                if k == 1:
                    g_.tensor_tensor(
                        sclp[:, 2048:].rearrange("q (m h d) -> q m h d",
                                                 m=1, h=HEADS),
                        vp[:, 2048:].rearrange("q (m h d) -> q m h d",
                                               m=1, h=HEADS),
                        attnp4[:, 8:9, :, None].to_broadcast(
                            (B, 1, HEADS, DH)),
                        mybir.AluOpType.mult).then_inc(pmul, 1)
                    continue
                p, i = divmod(k - 2, 9)
                m0 = 8 * i
                mw = 8 if i < 8 else 1
                if i == 0 and p >= 2:
                    # lvl1/osl[p%2] reuse: DVE finished pair p-2
                    g_.wait_ge(dself, 8 * (p - 2) + 9)
                if i < 8:
                    li = 8 * p + i          # global L1-add index
                    if li >= 2:
                        g_.wait_ge(padd, li - 1)      # scr[i%2] free (DVE)
                    out_ap = scr[i % 2][:].rearrange(
                        "q (m h d) -> q m h d", m=mw, h=HEADS)
                else:
                    out_ap = osl[p % 2][:].rearrange(
                        "q (m h d) -> q m h d", m=mw, h=HEADS)
                g_.tensor_tensor(
                    out_ap,
                    vrow[k % 4][:, :mw * INNER].rearrange(
                        "q (m h d) -> q m h d", m=mw, h=HEADS),
                    attn4[:, p, m0:m0 + mw, :, None].to_broadcast(
                        (2 * B, mw, HEADS, DH)),
                    mybir.AluOpType.mult).then_inc(pmul, 1)


        # ---- DVE: partial reduce + bf16 fold tree per pair ----------------
        # dself incs: 1 (partial) + 8 per pair.
        @block.vector
        def _(v):
            v.wait_ge(pmul, 2)
            v.tensor_reduce(
                accp[:].rearrange("q (h d) -> q h d", h=HEADS),
                sclp[:].rearrange("q (m h d) -> q h d m", m=9, h=HEADS),
                axis=mybir.AxisListType.X,
                op=mybir.AluOpType.add).then_inc(dself, 1)
            for p in range(NPAIR):
                base = 1 + 8 * p
                pl = p % 2
                if p >= 2:
                    # all out DMAs issued so far (outp + pairs 0..p-1) done
                    v.wait_ge(osem, 16 * (p + 1))
                # L1: fold each pool-mult group (scr) into lvl1 slabs
                for i in range(8):
                    v.wait_ge(pmul, 3 + 9 * p + i)    # group mult done
                    if i == 0:
                        v.wait_ge(dself, base)        # lvl1/ta/ub[pl] free
                    v.tensor_tensor(
                        lvl1[pl][:, 1024 * i:1024 * (i + 1)],
                        scr[i % 2][:, :1024], scr[i % 2][:, 1024:],
                        mybir.AluOpType.add).then_inc(padd, 1)
                    if i == 3:
                        # slabs 0..3 ready: start the left fold
                        v.wait_ge(padd, 8 * p + 4)
                        v.tensor_tensor(ta[pl][:], lvl1[pl][:, :2048],
                                        lvl1[pl][:, 2048:4096],
                                        mybir.AluOpType.add).then_inc(
                                            dself, 1)
                        v.wait_ge(dself, base + 1)
                        v.tensor_tensor(ub[pl][0][:], ta[pl][:, :1024],
                                        ta[pl][:, 1024:],
                                        mybir.AluOpType.add).then_inc(
                                            dself, 1)
                v.wait_ge(padd, 8 * p + 8)            # slabs 4..7 in lvl1
                v.wait_ge(dself, base + 2)
                v.tensor_tensor(ta[pl][:], lvl1[pl][:, 4096:6144],
                                lvl1[pl][:, 6144:8192],
                                mybir.AluOpType.add).then_inc(dself, 1)
                v.wait_ge(dself, base + 3)
                v.tensor_tensor(ub[pl][1][:], ta[pl][:, :1024],
                                ta[pl][:, 1024:],
                                mybir.AluOpType.add).then_inc(dself, 1)
                v.wait_ge(dself, base + 4)
                v.tensor_tensor(ta[pl][:, :1024], ub[pl][0][:], ub[pl][1][:],
                                mybir.AluOpType.add).then_inc(dself, 1)
                v.wait_ge(dself, base + 5)
                v.tensor_tensor(ub[pl][0][:, :512], ta[pl][:, :512],
                                ta[pl][:, 512:1024],
                                mybir.AluOpType.add).then_inc(dself, 1)
                v.wait_ge(dself, base + 6)
                v.tensor_tensor(ta[pl][:, :256], ub[pl][0][:, :256],
                                ub[pl][0][:, 256:512],
                                mybir.AluOpType.add).then_inc(dself, 1)
                v.wait_ge(pmul, 2 + 9 * p + 9)        # odd-slice mult done
                v.wait_ge(dself, base + 7)
                v.tensor_tensor(acc2[pl][:], ta[pl][:, :256], osl[pl][:],
                                mybir.AluOpType.add).then_inc(dself, 1)

    return nc


def _to_bf16(a):
    import ml_dtypes
    return np.asarray(a, dtype=ml_dtypes.bfloat16)


def host_prep(x, gamma, beta, Wqk, Wv, Wout, bout):
    """LayerNorm + qk + softmax on host; build per-core in_maps."""
    x = np.asarray(x, np.float32)
    mu = x.mean(-1, keepdims=True)
    var = np.square(x - mu).mean(-1, keepdims=True)
    xn = ((x - mu) / np.sqrt(var + EPS) * np.asarray(gamma, np.float32)
          + np.asarray(beta, np.float32)).astype(np.float32)

    qk = xn @ np.asarray(Wqk, np.float32)
    q, k = qk[..., :INNER], qk[..., INNER:]
    q = q.reshape(B, N, HEADS, DH).transpose(0, 2, 1, 3)
    k = k.reshape(B, N, HEADS, DH).transpose(0, 2, 1, 3)
    dots = np.einsum("bhnd,bhmd->bhnm", q, k) * (DH ** -0.5)
    dots -= dots.max(-1, keepdims=True)
    e = np.exp(dots)
    attn = (e / e.sum(-1, keepdims=True)).astype(np.float32)  # [b,h,n,m]

    # [n, d, m*e] bf16 weight stream source
    WvT = np.ascontiguousarray(
        _to_bf16(Wv).transpose(0, 2, 1, 3).reshape(N, DIM, N * INNER))
    xnT = _to_bf16(xn.transpose(2, 1, 0))       # [d, n, b]

    in_maps = []
    for c in range(8):
        rows = [8 * c + i for i in range(8)]
        # wv stream: rows in natural order [A0|B0|A1|B1|...]; quarter
        # granules address strided slices of this layout directly.
        wv_g = np.empty((DIM, 2 * NPAIR * ROW_COLS), WvT.dtype)
        for p in range(NPAIR):
            wv_g[:, (2 * p) * ROW_COLS:(2 * p + 1) * ROW_COLS] = \
                WvT[rows[2 * p]]
            wv_g[:, (2 * p + 1) * ROW_COLS:(2 * p + 2) * ROW_COLS] = \
                WvT[rows[2 * p + 1]]
        # partial row m-range
        mstart, mcount = 8 * c, (9 if c == 7 else 8)
        wvp_c = np.zeros((DIM, P_COLS), WvT.dtype)
        wvp_c[:, :mcount * INNER] = WvT[64][
            :, mstart * INNER:(mstart + mcount) * INNER]
        # activations / attention
        xnT2_c = np.empty((DIM, NPAIR, 2 * B), xnT.dtype)
        attn2_c = np.empty((2 * B, NPAIR, N, HEADS), np.float32)
        for p in range(NPAIR):
            for par in range(2):
                nrow = rows[2 * p + par]
                xnT2_c[:, p, par * B:(par + 1) * B] = xnT[:, nrow, :]
                attn2_c[par * B:(par + 1) * B, p] = \
                    attn[:, :, nrow, :].transpose(0, 2, 1)
        attnp_c = np.zeros((B, 9, HEADS), np.float32)
        attnp_c[:, :mcount] = attn[
            :, :, 64, mstart:mstart + mcount].transpose(0, 2, 1)
        in_maps.append({"wv": wv_g, "wvp": wvp_c, "xnT2": xnT2_c,
                        "xnTp": _to_bf16(xn[:, 64, :].T),
                        "attn2": _to_bf16(attn2_c),
                        "attnp": _to_bf16(attnp_c)})
    return in_maps, xn, attn


def assemble(results, Wout, bout):
    out_pre = np.zeros((B, N, INNER), np.float32)
    for c in range(8):
        o2 = np.asarray(results[c]["out2"], np.float32)  # [2B, NPAIR, INNER]
        for p in range(NPAIR):
            out_pre[:, 8 * c + 2 * p, :] = o2[:B, p]
            out_pre[:, 8 * c + 2 * p + 1, :] = o2[B:, p]
        out_pre[:, 64, :] += np.asarray(results[c]["outp"], np.float32)
    out = out_pre.reshape(B * N, INNER) @ np.asarray(Wout, np.float32) \
        + np.asarray(bout, np.float32)
    return out.reshape(B, N, DIM).astype(np.float32)


def kernel(x, gamma, beta, Wqk, Wv, Wout, bout):
    in_maps, _, _ = host_prep(x, gamma, beta, Wqk, Wv, Wout, bout)
    if "nc" not in _CACHED:
        _CACHED["nc"] = _build_program()
    res = run_bass_kernel_spmd(_CACHED["nc"], in_maps, list(range(8))).results
    return assemble(res, Wout, bout)


# revision 40
# speedup vs baseline: 1.0428x; 1.0230x over previous
"""Trainium2 kernel for nn_Attention_50182397886533.

Computation: LayerNorm + q/k + softmax on host (<3% of FLOPs); device
computes the dominant per-(query,key) value projection
    v[b,n,m,:] = xn[b,n,:] @ Wv[n,m]            (Wv: [65,65,128,256])
fused with the attention-weighted reduction over keys m.  Host applies
the small final Wout projection.

Sharding: query rows n across 8 cores — 8 full rows per core, plus the
straggler row 64 split over m (8 keys/core, core 7 gets 9).

Per-core device pipeline (DMA-bound at ~360 GB/s, ~108us stream):
  - Wv streamed as bf16 row-piece granules through a 10-slot SBUF ring
    issued by the sync engine (SP); piece sizes shrink toward the end of
    each row so the serial drain tail stays short.
  - Rows processed in PAIRS: each matmul chunk (2 keys x 256) for row A
    goes to PSUM partitions 0:64, row B to 64:128 of the same 4-bank
    half, so all post-matmul work runs at full 128-partition width.
  - ACT (scalar) drains PSUM -> SBUF as bf16 1024-wide copies (GPSIMD
    cannot access PSUM; DVE reads of PSUM get no fast modes).
  - Pool (gpsimd) applies the attention weights: one 8-key-wide
    tensor_tensor multiply per drained group, SBUF-only.
  - DVE sums over keys with a bf16 binary fold tree (packed bf16 adds
    get the DVE 2x mode; tensor_reduce would run at 1x), interleaved
    with pool progress, then ships acc via ACT-issued output DMAs.
"""

import numpy as np

import concourse.bass as bass
import concourse.mybir as mybir
from concourse.bass_utils import run_bass_kernel_spmd

B = 64
N = 65
DIM = 128
HEADS = 8
DH = 32
INNER = 256
EPS = 1e-5

NPAIR = 4            # row pairs per core
NCHUNK = 33          # m-chunks per row (32 of width 2 + 1 of width 1)
ROW_COLS = N * INNER           # 16640 bf16 cols per row stream
NPCH = 5             # partial-row chunks (9 m slots: 4x2+1)
P_COLS = 9 * INNER             # partial-row stream cols (2304)

# chunk tables: (m0, mw) per j
CHUNKS = [(2 * j, 2) for j in range(32)] + [(64, 1)]
PCHUNKS = [(2 * j, 2) for j in range(4)] + [(8, 1)]

# Rows are streamed in QUARTERS; 8 granules per pair in consumption
# order A-q0, B-q0, A-q1, B-q1, ... Quarter q covers chunks JQ[q].
# weighted so the LAST quarter (the serial tail) is smallest
# DMA pieces per row (last ones small: they gate the serial drain tail)
JQ = [(0, 10), (10, 20), (20, 28), (28, 31), (31, 33)]
Q_OFF = [0, 5120, 10240, 14336, 15872]           # col offset of piece in row
Q_LEN = [5120, 5120, 4096, 1536, 768]            # cols per piece
NQ = len(JQ)
NSLOTS = 2 * NQ                # ring slots: one per piece-granule of a pair


# Pool owns all multiply chunks; DVE owns the reductions.

_CACHED = {}


def _granule(g):
    """Granule g -> (dram col start, length). 2*NQ per pair: A/B per piece."""
    p, k = divmod(g, 2 * NQ)
    q, par = divmod(k, 2)
    base = p * 2 * ROW_COLS + par * ROW_COLS
    return base + Q_OFF[q], Q_LEN[q]


def _build_program():
    nc = bass.Bass()
    f32, bf16 = mybir.dt.float32, mybir.dt.bfloat16

    wv = nc.dram_tensor("wv", [DIM, 2 * NPAIR * ROW_COLS], bf16,
                        kind="ExternalInput")
    # header blob: xnT2 | xnTp | wvp | attn2 | attnp packed column-wise
    HDR_COLS = 512 + 64 + P_COLS + NPAIR * N * HEADS + 9 * HEADS
    XO, XPO = 0, 512
    WO = 576
    AO = WO + P_COLS
    PO = AO + NPAIR * N * HEADS
    hdr = nc.dram_tensor("hdr", [DIM, HDR_COLS], bf16, kind="ExternalInput")
    out2 = nc.dram_tensor("out2", [2 * B, NPAIR, INNER], f32,
                          kind="ExternalOutput")
    outp = nc.dram_tensor("outp", [B, INNER], f32, kind="ExternalOutput")

    # build-time schedules -------------------------------------------------
    # global chunk order: partial-row chunks first (G 0..4), then pairs
    # (G = NPCH + NCHUNK*p + j).  Pool owns every multiply in this order.
    NG = NPCH + NPAIR * NCHUNK                # 137
    # matmul count after chunk G fully issued (pair chunks have 2 matmuls)
    mm_after = {}
    mm = 0
    for G in range(NG):
        mm += 1 if G < NPCH else 2
        mm_after[G] = mm
    # 4-chunk groups: partial row = groups 0,1; pair p group i (i<9) is
    # global group 2 + 9p + i (8 full groups of 4 chunks + chunk 32 alone).
    NGRP = 2 + 9 * NPAIR
    def grp_chunks(k):
        if k == 0:
            return list(range(4))
        if k == 1:
            return [4]
        p, i = divmod(k - 2, 9)
        j0 = 4 * i
        return [NPCH + NCHUNK * p + j for j in range(j0, min(j0 + 4, NCHUNK))]
    mm_after_grp = [mm_after[grp_chunks(k)[-1]] for k in range(NGRP)]
    acount = []
    _a = 0
    for k in range(NGRP):
        _a += 2 if len(grp_chunks(k)) == 4 else 1
        acount.append(_a)

    import contextlib
    with contextlib.ExitStack() as st:
        ring = [st.enter_context(nc.sbuf_tensor(f"ring{s}",
                                                [DIM, Q_LEN[s // 2]], bf16))
                for s in range(NSLOTS)]
        hdr_sb = st.enter_context(nc.sbuf_tensor([DIM, HDR_COLS], bf16))
        # PSUM: two 4-bank halves; groups alternate between them
        psA = st.enter_context(nc.psum_tensor([2 * B, 2048], f32))
        psB = st.enter_context(nc.psum_tensor([2 * B, 2048], f32))
        # ACT drain targets: 4-slot ring of raw v (bf16), partial-row vp
        vrow = [st.enter_context(nc.sbuf_tensor(f"vrow{s}", [2 * B, 2048],
                                                bf16)) for s in range(4)]
        vp = st.enter_context(nc.sbuf_tensor([B, P_COLS], bf16))
        # pool mult outputs: 2-slot scratch; L1 pair-sums; odd m=64 slice
        scr = [st.enter_context(nc.sbuf_tensor(f"scr{s}", [2 * B, 2048],
                                               bf16)) for s in range(2)]
        racc = [st.enter_context(nc.sbuf_tensor(f"racc{s}", [2 * B, 1024],
                                                bf16)) for s in range(2)]
        tb = [st.enter_context(nc.sbuf_tensor(f"tb{s}", [2 * B, 1024], bf16))
              for s in range(2)]
        osl = [st.enter_context(nc.sbuf_tensor(f"osl{s}", [2 * B, INNER],
                                               bf16)) for s in range(2)]
        sclp = st.enter_context(nc.sbuf_tensor([B, P_COLS], bf16))
        # DVE fold temps
        ta = [st.enter_context(nc.sbuf_tensor(f"ta{s}", [2 * B, 2048], bf16))
              for s in range(2)]
        ub = [[st.enter_context(nc.sbuf_tensor(f"ub{s}_{q}", [2 * B, 1024],
                                               bf16)) for q in range(2)]
              for s in range(2)]
        acc2 = [st.enter_context(nc.sbuf_tensor(f"acc2{s}", [2 * B, INNER],
                                                f32)) for s in range(2)]
        accp = st.enter_context(nc.sbuf_tensor([B, INNER], f32))

        hdrx = st.enter_context(nc.semaphore("hdrx"))
        hdra = st.enter_context(nc.semaphore("hdra"))
        gsem = [st.enter_context(nc.semaphore(f"gsem{s}"))
                for s in range(NSLOTS)]
        gpsem = st.enter_context(nc.semaphore("gpsem"))
        pe_mm = st.enter_context(nc.semaphore("pe_mm"))
        asem = st.enter_context(nc.semaphore("asem"))
        pmul = st.enter_context(nc.semaphore("pmul"))
        padd = st.enter_context(nc.semaphore("padd"))
        dself = st.enter_context(nc.semaphore("dself"))
        osem = st.enter_context(nc.semaphore("osem"))
        block = st.enter_context(nc.Block())

        # chunk -> (group, position) map
        chunk_grp = {}
        for k in range(NGRP):
            for pos, G in enumerate(grp_chunks(k)):
                chunk_grp[G] = (k, pos)
        ps = [psA, psB]

        # ---- sync engine: input DMA stream --------------------------------
        @block.sync
        def _(sy):
            sy.dma_start(hdr_sb[:], hdr.ap()).then_inc(hdrx, 16)
            for g in range(2 * NQ * NPAIR):
                if g >= NSLOTS:
                    # slot reuse: matmuls consuming the same piece of the
                    # previous pair must be done
                    gp, kp = divmod(g - NSLOTS, NSLOTS)
                    jlast = JQ[kp // 2][1] - 1
                    sy.wait_ge(pe_mm, mm_after[NPCH + NCHUNK * gp + jlast])
                c0, clen = _granule(g)
                sy.dma_start(ring[g % NSLOTS][:, :clen],
                             wv.ap()[:, c0:c0 + clen]).then_inc(
                                 gsem[g % NSLOTS], 16)

        # ---- tensor engine: value-projection matmuls ----------------------
        @block.tensor
        def _(t):
            t.wait_ge(hdrx, 16)         # header blob loaded
            for j5 in range(NPCH):
                k, pos = chunk_grp[j5]
                m0, mw = PCHUNKS[j5]
                cols = mw * INNER
                t.matmul(ps[k % 2][0:B, 512 * pos:512 * pos + cols],
                         hdr_sb[:, XPO:XPO + B],
                         hdr_sb[:, WO + 512 * j5:WO + 512 * j5 + cols],
                         start=True, stop=True).then_inc(pe_mm, 1)
            for p in range(NPAIR):
                for j in range(NCHUNK):
                    G = NPCH + NCHUNK * p + j
                    k, pos = chunk_grp[G]
                    q = next(i for i, (a, b) in enumerate(JQ) if a <= j < b)
                    ga, gb = NSLOTS * p + 2 * q, NSLOTS * p + 2 * q + 1
                    if pos == 0:
                        # psum half reuse: group k-2's drain copies done
                        t.wait_ge(asem, acount[k - 2])
                    m0, mw = CHUNKS[j]
                    cols = mw * INNER
                    off = 512 * (j - JQ[q][0])
                    bank = ps[k % 2]
                    if j == JQ[q][0]:
                        t.wait_ge(gsem[ga % NSLOTS], 16 * (ga // NSLOTS + 1))
                    t.matmul(bank[0:B, 512 * pos:512 * pos + cols],
                             hdr_sb[:, XO + p * 2 * B:XO + p * 2 * B + B],
                             ring[ga % NSLOTS][:, off:off + cols],
                             start=True, stop=True).then_inc(pe_mm, 1)
                    if j == JQ[q][0]:
                        t.wait_ge(gsem[gb % NSLOTS], 16 * (gb // NSLOTS + 1))
                    t.matmul(bank[B:2 * B, 512 * pos:512 * pos + cols],
                             hdr_sb[:, XO + p * 2 * B + B:XO + (p + 1) * 2 * B],
                             ring[gb % NSLOTS][:, off:off + cols],
                             start=True, stop=True).then_inc(pe_mm, 1)

        # ---- ACT: PSUM -> SBUF drain copies (bf16) + output DMAs ----------
        @block.scalar
        def _(s):
            for k in range(NGRP):
                if k >= 2 and len(grp_chunks(k)) == 4:
                    # vrow slot reuse: pool mult of group k-4 done
                    if k >= 6:
                        s.wait_ge(pmul, k - 3)
                    dst = vp if k == 0 else vrow[k % 4]
                    # two half-copies so the drain interleaves finer
                    s.wait_ge(pe_mm, mm_after[grp_chunks(k)[1]])
                    s.copy(dst[:, :1024],
                           ps[k % 2][:, :1024]).then_inc(asem, 1)
                    s.wait_ge(pe_mm, mm_after_grp[k])
                    s.copy(dst[:, 1024:2048],
                           ps[k % 2][:, 1024:2048]).then_inc(asem, 1)
                elif k == 0:
                    s.wait_ge(pe_mm, mm_after[grp_chunks(0)[1]])
                    s.copy(vp[:, :1024], ps[0][0:B, :1024]).then_inc(asem, 1)
                    s.wait_ge(pe_mm, mm_after_grp[0])
                    s.copy(vp[:, 1024:2048],
                           ps[0][0:B, 1024:2048]).then_inc(asem, 1)
                elif k == 1:
                    s.wait_ge(pe_mm, mm_after_grp[1])
                    s.copy(vp[:, 2048:], ps[1][0:B, :256]).then_inc(asem, 1)
                else:
                    p, i = divmod(k - 2, 9)
                    s.wait_ge(pe_mm, mm_after_grp[k])
                    if k >= 6:
                        s.wait_ge(pmul, k - 3)
                    s.copy(vrow[k % 4][:, :256],
                           ps[k % 2][:, :256]).then_inc(asem, 1)
                if k == 2:
                    s.wait_ge(dself, 1)
                    s.dma_start(outp.ap()[:], accp[:]).then_inc(osem, 16)
                if k >= 11 and (k - 2) % 9 == 0:
                    # pair (k-2)//9 - 1 finished: ship it
                    pq = (k - 2) // 9 - 1
                    s.wait_ge(dself, 10 * pq + 11)
                    s.dma_start(out2.ap()[:, pq, :],
                                acc2[pq % 2][:]).then_inc(osem, 16)
            s.wait_ge(dself, 10 * (NPAIR - 1) + 11)
            s.dma_start(out2.ap()[:, NPAIR - 1, :],
                        acc2[(NPAIR - 1) % 2][:]).then_inc(osem, 16)

        # ---- Pool: attention multiplies + first-level pair sums -----------
        @block.gpsimd
        def _(g_):
            g_.wait_ge(hdrx, 16)
            attn4 = hdr_sb[:, AO:AO + NPAIR * N * HEADS].rearrange(
                "q (p m h) -> q p m h", p=NPAIR, m=N)
            attnp4 = hdr_sb[0:B, PO:PO + 9 * HEADS].rearrange(
                "q (m h) -> q m h", m=9)
            for k in range(NGRP):
                g_.wait_ge(asem, acount[k])
                if k == 0:
                    g_.tensor_tensor(
                        sclp[:, :2048].rearrange("q (m h d) -> q m h d",
                                                 m=8, h=HEADS),
                        vp[:, :2048].rearrange("q (m h d) -> q m h d",
                                               m=8, h=HEADS),
                        attnp4[:, 0:8, :, None].to_broadcast(
                            (B, 8, HEADS, DH)),
                        mybir.AluOpType.mult).then_inc(pmul, 1)
                    continue
                if k == 1:
                    g_.tensor_tensor(
                        sclp[:, 2048:].rearrange("q (m h d) -> q m h d",
                                                 m=1, h=HEADS),
                        vp[:, 2048:].rearrange("q (m h d) -> q m h d",
                                               m=1, h=HEADS),
                        attnp4[:, 8:9, :, None].to_broadcast(
                            (B, 1, HEADS, DH)),
                        mybir.AluOpType.mult).then_inc(pmul, 1)
                    continue
                p, i = divmod(k - 2, 9)
                m0 = 8 * i
                mw = 8 if i < 8 else 1
                if i == 0 and p >= 2:
                    # racc/osl[p%2] reuse: DVE finished pair p-2
                    g_.wait_ge(dself, 10 * (p - 2) + 11)
                if i < 8:
                    li = 8 * p + i          # global L1-add index
                    if li >= 2:
                        g_.wait_ge(padd, li - 1)      # scr[i%2] free (DVE)
                    out_ap = scr[i % 2][:].rearrange(
                        "q (m h d) -> q m h d", m=mw, h=HEADS)
                else:
                    out_ap = osl[p % 2][:].rearrange(
                        "q (m h d) -> q m h d", m=mw, h=HEADS)
                g_.tensor_tensor(
                    out_ap,
                    vrow[k % 4][:, :mw * INNER].rearrange(
                        "q (m h d) -> q m h d", m=mw, h=HEADS),
                    attn4[:, p, m0:m0 + mw, :, None].to_broadcast(
                        (2 * B, mw, HEADS, DH)),
                    mybir.AluOpType.mult).then_inc(pmul, 1)


        # ---- DVE: partial reduce + bf16 fold tree per pair ----------------
        # dself incs: 1 (partial) + 8 per pair.
        @block.vector
        def _(v):
            v.wait_ge(pmul, 2)
            v.tensor_reduce(
                accp[:].rearrange("q (h d) -> q h d", h=HEADS),
                sclp[:].rearrange("q (m h d) -> q h d m", m=9, h=HEADS),
                axis=mybir.AxisListType.X,
                op=mybir.AluOpType.add).then_inc(dself, 1)
            dcnt = 1                      # dself value so far
            for p in range(NPAIR):
                pl = p % 2
                if p >= 2:
                    # all out DMAs issued so far (outp + pairs 0..p-1) done
                    v.wait_ge(osem, 16 * (p + 1))
                for i in range(8):
                    v.wait_ge(pmul, 3 + 9 * p + i)    # group mult done
                    v.wait_ge(dself, dcnt)            # serialize the chain
                    if i == 0:
                        v.tensor_tensor(racc[pl][:], scr[0][:, :1024],
                                        scr[0][:, 1024:],
                                        mybir.AluOpType.add).then_inc(
                                            padd, 1)
                    else:
                        v.tensor_tensor(tb[pl][:], scr[i % 2][:, :1024],
                                        scr[i % 2][:, 1024:],
                                        mybir.AluOpType.add).then_inc(
                                            padd, 1)
                        v.wait_ge(padd, 8 * p + i + 1)
                        v.tensor_tensor(racc[pl][:], racc[pl][:], tb[pl][:],
                                        mybir.AluOpType.add).then_inc(
                                            dself, 1)
                        dcnt += 1
                v.wait_ge(dself, dcnt)
                v.tensor_tensor(ub[pl][0][:, :512], racc[pl][:, :512],
                                racc[pl][:, 512:],
                                mybir.AluOpType.add).then_inc(dself, 1)
                dcnt += 1
                v.wait_ge(dself, dcnt)
                v.tensor_tensor(ta[pl][:, :256], ub[pl][0][:, :256],
                                ub[pl][0][:, 256:512],
                                mybir.AluOpType.add).then_inc(dself, 1)
                dcnt += 1
                v.wait_ge(pmul, 2 + 9 * p + 9)        # odd-slice mult done
                v.wait_ge(dself, dcnt)
                v.tensor_tensor(acc2[pl][:], ta[pl][:, :256], osl[pl][:],
                                mybir.AluOpType.add).then_inc(dself, 1)
                dcnt += 1
    return nc


def _to_bf16(a):
    import ml_dtypes
    return np.asarray(a, dtype=ml_dtypes.bfloat16)


def host_prep(x, gamma, beta, Wqk, Wv, Wout, bout):
    """LayerNorm + qk + softmax on host; build per-core in_maps."""
    x = np.asarray(x, np.float32)
    mu = x.mean(-1, keepdims=True)
    var = np.square(x - mu).mean(-1, keepdims=True)
    xn = ((x - mu) / np.sqrt(var + EPS) * np.asarray(gamma, np.float32)
          + np.asarray(beta, np.float32)).astype(np.float32)

    qk = xn @ np.asarray(Wqk, np.float32)
    q, k = qk[..., :INNER], qk[..., INNER:]
    q = q.reshape(B, N, HEADS, DH).transpose(0, 2, 1, 3)
    k = k.reshape(B, N, HEADS, DH).transpose(0, 2, 1, 3)
    dots = np.einsum("bhnd,bhmd->bhnm", q, k) * (DH ** -0.5)
    dots -= dots.max(-1, keepdims=True)
    e = np.exp(dots)
    attn = (e / e.sum(-1, keepdims=True)).astype(np.float32)  # [b,h,n,m]

    # [n, d, m*e] bf16 weight stream source
    WvT = np.ascontiguousarray(
        _to_bf16(Wv).transpose(0, 2, 1, 3).reshape(N, DIM, N * INNER))
    xnT = _to_bf16(xn.transpose(2, 1, 0))       # [d, n, b]

    in_maps = []
    for c in range(8):
        rows = [8 * c + i for i in range(8)]
        # wv stream: rows in natural order [A0|B0|A1|B1|...]; quarter
        # granules address strided slices of this layout directly.
        wv_g = np.empty((DIM, 2 * NPAIR * ROW_COLS), WvT.dtype)
        for p in range(NPAIR):
            wv_g[:, (2 * p) * ROW_COLS:(2 * p + 1) * ROW_COLS] = \
                WvT[rows[2 * p]]
            wv_g[:, (2 * p + 1) * ROW_COLS:(2 * p + 2) * ROW_COLS] = \
                WvT[rows[2 * p + 1]]
        # partial row m-range
        mstart, mcount = 8 * c, (9 if c == 7 else 8)
        wvp_c = np.zeros((DIM, P_COLS), WvT.dtype)
        wvp_c[:, :mcount * INNER] = WvT[64][
            :, mstart * INNER:(mstart + mcount) * INNER]
        # activations / attention
        xnT2_c = np.empty((DIM, NPAIR, 2 * B), xnT.dtype)
        attn2_c = np.empty((2 * B, NPAIR, N, HEADS), np.float32)
        for p in range(NPAIR):
            for par in range(2):
                nrow = rows[2 * p + par]
                xnT2_c[:, p, par * B:(par + 1) * B] = xnT[:, nrow, :]
                attn2_c[par * B:(par + 1) * B, p] = \
                    attn[:, :, nrow, :].transpose(0, 2, 1)
        attnp_c = np.zeros((B, 9, HEADS), np.float32)
        attnp_c[:, :mcount] = attn[
            :, :, 64, mstart:mstart + mcount].transpose(0, 2, 1)
        import ml_dtypes
        hdr_c = np.zeros((DIM, 512 + 64 + P_COLS + NPAIR * N * HEADS
                          + 9 * HEADS), ml_dtypes.bfloat16)
        hdr_c[:, 0:512] = _to_bf16(xnT2_c.reshape(DIM, 512))
        hdr_c[:, 512:576] = _to_bf16(xn[:, 64, :].T)
        hdr_c[:, 576:576 + P_COLS] = wvp_c
        ao = 576 + P_COLS
        hdr_c[:, ao:ao + NPAIR * N * HEADS] = _to_bf16(
            attn2_c.reshape(2 * B, NPAIR * N * HEADS))
        hdr_c[0:B, ao + NPAIR * N * HEADS:] = _to_bf16(
            attnp_c.reshape(B, 9 * HEADS))
        in_maps.append({"wv": wv_g, "hdr": hdr_c})
    return in_maps, xn, attn


def assemble(results, Wout, bout):
    out_pre = np.zeros((B, N, INNER), np.float32)
    for c in range(8):
        o2 = np.asarray(results[c]["out2"], np.float32)  # [2B, NPAIR, INNER]
        for p in range(NPAIR):
            out_pre[:, 8 * c + 2 * p, :] = o2[:B, p]
            out_pre[:, 8 * c + 2 * p + 1, :] = o2[B:, p]
        out_pre[:, 64, :] += np.asarray(results[c]["outp"], np.float32)
    out = out_pre.reshape(B * N, INNER) @ np.asarray(Wout, np.float32) \
        + np.asarray(bout, np.float32)
    return out.reshape(B, N, DIM).astype(np.float32)


def kernel(x, gamma, beta, Wqk, Wv, Wout, bout):
    in_maps, _, _ = host_prep(x, gamma, beta, Wqk, Wv, Wout, bout)
    if "nc" not in _CACHED:
        _CACHED["nc"] = _build_program()
    res = run_bass_kernel_spmd(_CACHED["nc"], in_maps, list(range(8))).results
    return assemble(res, Wout, bout)


# revision 48
# speedup vs baseline: 1.0515x; 1.0084x over previous
"""Trainium2 kernel for nn_Attention_50182397886533.

Computation: LayerNorm + q/k + softmax on host (<3% of FLOPs); device
computes the dominant per-(query,key) value projection
    v[b,n,m,:] = xn[b,n,:] @ Wv[n,m]            (Wv: [65,65,128,256])
fused with the attention-weighted reduction over keys m.  Host applies
the small final Wout projection.

Sharding: query rows n across 8 cores — 8 full rows per core, plus the
straggler row 64 split over m (8 keys/core, core 7 gets 9).

Per-core device pipeline (DMA-bound at ~360 GB/s, ~108us stream):
  - Wv streamed as bf16 row-piece granules through a 10-slot SBUF ring
    issued by the sync engine (SP); piece sizes shrink toward the end of
    each row so the serial drain tail stays short.
  - Rows processed in PAIRS: each matmul chunk (2 keys x 256) for row A
    goes to PSUM partitions 0:64, row B to 64:128 of the same 4-bank
    half, so all post-matmul work runs at full 128-partition width.
  - ACT (scalar) drains PSUM -> SBUF as bf16 1024-wide copies (GPSIMD
    cannot access PSUM; DVE reads of PSUM get no fast modes).
  - Pool (gpsimd) applies the attention weights: one 8-key-wide
    tensor_tensor multiply per drained group, SBUF-only.
  - DVE sums over keys with a bf16 binary fold tree (packed bf16 adds
    get the DVE 2x mode; tensor_reduce would run at 1x), interleaved
    with pool progress, then ships acc via ACT-issued output DMAs.
"""

import numpy as np

import concourse.bass as bass
import concourse.mybir as mybir
from concourse.bass_utils import run_bass_kernel_spmd

B = 64
N = 65
DIM = 128
HEADS = 8
DH = 32
INNER = 256
EPS = 1e-5

NPAIR = 4            # row pairs per core
NCHUNK = 33          # m-chunks per row (32 of width 2 + 1 of width 1)
ROW_COLS = N * INNER           # 16640 bf16 cols per row stream
NPCH = 5             # partial-row chunks (9 m slots: 4x2+1)
P_COLS = 9 * INNER             # partial-row stream cols (2304)

# chunk tables: (m0, mw) per j
CHUNKS = [(2 * j, 2) for j in range(32)] + [(64, 1)]
PCHUNKS = [(2 * j, 2) for j in range(4)] + [(8, 1)]

# Rows are streamed in QUARTERS; 8 granules per pair in consumption
# order A-q0, B-q0, A-q1, B-q1, ... Quarter q covers chunks JQ[q].
# weighted so the LAST quarter (the serial tail) is smallest
# DMA pieces per row (last ones small: they gate the serial drain tail)
JQ = [(0, 10), (10, 20), (20, 28), (28, 33)]
Q_OFF = [0, 5120, 10240, 14336]                  # col offset of piece in row
Q_LEN = [5120, 5120, 4096, 2304]                 # cols per piece
NQ = len(JQ)
NSLOTS = 2 * NQ                # ring slots: one per piece-granule of a pair


# Pool owns all multiply chunks; DVE owns the reductions.

_CACHED = {}


def _granule(g):
    """Granule g -> (dram col start, length). 2*NQ per pair: A/B per piece."""
    p, k = divmod(g, 2 * NQ)
    q, par = divmod(k, 2)
    base = p * 2 * ROW_COLS + par * ROW_COLS
    return base + Q_OFF[q], Q_LEN[q]


def _build_program():
    nc = bass.Bass()
    f32, bf16 = mybir.dt.float32, mybir.dt.bfloat16

    wv = nc.dram_tensor("wv", [DIM, 2 * NPAIR * ROW_COLS], bf16,
                        kind="ExternalInput")
    # header blob: xnT2 | xnTp | wvp | attn2 | attnp packed column-wise
    HDR_COLS = 512 + 64 + P_COLS + NPAIR * N * HEADS + 9 * HEADS
    XO, XPO = 0, 512
    WO = 576
    AO = WO + P_COLS
    PO = AO + NPAIR * N * HEADS
    hdr = nc.dram_tensor("hdr", [DIM, HDR_COLS], bf16, kind="ExternalInput")
    out2 = nc.dram_tensor("out2", [2 * B, NPAIR, INNER], bf16,
                          kind="ExternalOutput")
    outp = nc.dram_tensor("outp", [B, INNER], bf16, kind="ExternalOutput")

    # build-time schedules -------------------------------------------------
    # global chunk order: partial-row chunks first (G 0..4), then pairs
    # (G = NPCH + NCHUNK*p + j).  Pool owns every multiply in this order.
    NG = NPCH + NPAIR * NCHUNK                # 137
    # matmul count after chunk G fully issued (pair chunks have 2 matmuls)
    mm_after = {}
    mm = 0
    for G in range(NG):
        mm += 1 if G < NPCH else 2
        mm_after[G] = mm
    # 4-chunk groups: partial row = groups 0,1; pair p group i (i<9) is
    # global group 2 + 9p + i (8 full groups of 4 chunks + chunk 32 alone).
    NGRP = 2 + 9 * NPAIR
    def grp_chunks(k):
        if k == 0:
            return list(range(4))
        if k == 1:
            return [4]
        p, i = divmod(k - 2, 9)
        j0 = 4 * i
        return [NPCH + NCHUNK * p + j for j in range(j0, min(j0 + 4, NCHUNK))]
    mm_after_grp = [mm_after[grp_chunks(k)[-1]] for k in range(NGRP)]
    acount = []
    pm_after = []
    _a = _m = 0
    for k in range(NGRP):
        full = len(grp_chunks(k)) == 4
        _a += 2 if full else 1
        acount.append(_a)
        _m += 2 if (full and k >= 2) else 1
        pm_after.append(_m)

    import contextlib
    with contextlib.ExitStack() as st:
        ring = [st.enter_context(nc.sbuf_tensor(f"ring{s}",
                                                [DIM, Q_LEN[s // 2]], bf16))
                for s in range(NSLOTS)]
        hdr_sb = st.enter_context(nc.sbuf_tensor([DIM, HDR_COLS], bf16))
        # PSUM: two 4-bank halves; groups alternate between them
        psA = st.enter_context(nc.psum_tensor([2 * B, 2048], f32))
        psB = st.enter_context(nc.psum_tensor([2 * B, 2048], f32))
        # ACT drain targets: 4-slot ring of raw v (bf16), partial-row vp
        vrow = [st.enter_context(nc.sbuf_tensor(f"vrow{s}", [2 * B, 2048],
                                                bf16)) for s in range(4)]
        vp = st.enter_context(nc.sbuf_tensor([B, P_COLS], bf16))
        # pool mult outputs: 2-slot scratch; L1 pair-sums; odd m=64 slice
        scr = [st.enter_context(nc.sbuf_tensor(f"scr{s}", [2 * B, 2048],
                                               bf16)) for s in range(2)]
        racc = [st.enter_context(nc.sbuf_tensor(f"racc{s}", [2 * B, 1024],
                                                bf16)) for s in range(2)]
        tb = [st.enter_context(nc.sbuf_tensor(f"tb{s}", [2 * B, 1024], bf16))
              for s in range(2)]
        osl = [st.enter_context(nc.sbuf_tensor(f"osl{s}", [2 * B, INNER],
                                               bf16)) for s in range(2)]
        sclp = st.enter_context(nc.sbuf_tensor([B, P_COLS], bf16))
        # DVE fold temps
        ta = [st.enter_context(nc.sbuf_tensor(f"ta{s}", [2 * B, 2048], bf16))
              for s in range(2)]
        ub = [[st.enter_context(nc.sbuf_tensor(f"ub{s}_{q}", [2 * B, 1024],
                                               bf16)) for q in range(2)]
              for s in range(2)]
        acc2 = [st.enter_context(nc.sbuf_tensor(f"acc2{s}", [2 * B, INNER],
                                                bf16)) for s in range(2)]
        accp = st.enter_context(nc.sbuf_tensor([B, INNER], bf16))

        hdrx = st.enter_context(nc.semaphore("hdrx"))
        hdra = st.enter_context(nc.semaphore("hdra"))
        gsem = [st.enter_context(nc.semaphore(f"gsem{s}"))
                for s in range(NSLOTS)]
        gpsem = st.enter_context(nc.semaphore("gpsem"))
        pe_mm = st.enter_context(nc.semaphore("pe_mm"))
        asem = st.enter_context(nc.semaphore("asem"))
        pmul = st.enter_context(nc.semaphore("pmul"))
        padd = st.enter_context(nc.semaphore("padd"))
        dself = st.enter_context(nc.semaphore("dself"))
        osem = st.enter_context(nc.semaphore("osem"))
        block = st.enter_context(nc.Block())

        # chunk -> (group, position) map
        chunk_grp = {}
        for k in range(NGRP):
            for pos, G in enumerate(grp_chunks(k)):
                chunk_grp[G] = (k, pos)
        ps = [psA, psB]

        # ---- sync engine: input DMA stream --------------------------------
        @block.sync
        def _(sy):
            sy.dma_start(hdr_sb[:], hdr.ap()).then_inc(hdrx, 16)
            for g in range(2 * NQ * NPAIR):
                if g >= NSLOTS:
                    # slot reuse: matmuls consuming the same piece of the
                    # previous pair must be done
                    gp, kp = divmod(g - NSLOTS, NSLOTS)
                    jlast = JQ[kp // 2][1] - 1
                    sy.wait_ge(pe_mm, mm_after[NPCH + NCHUNK * gp + jlast])
                c0, clen = _granule(g)
                sy.dma_start(ring[g % NSLOTS][:, :clen],
                             wv.ap()[:, c0:c0 + clen]).then_inc(
                                 gsem[g % NSLOTS], 16)

        # ---- tensor engine: value-projection matmuls ----------------------
        @block.tensor
        def _(t):
            t.wait_ge(hdrx, 16)         # header blob loaded
            for j5 in range(NPCH):
                k, pos = chunk_grp[j5]
                m0, mw = PCHUNKS[j5]
                cols = mw * INNER
                t.matmul(ps[k % 2][0:B, 512 * pos:512 * pos + cols],
                         hdr_sb[:, XPO:XPO + B],
                         hdr_sb[:, WO + 512 * j5:WO + 512 * j5 + cols],
                         start=True, stop=True).then_inc(pe_mm, 1)
            for p in range(NPAIR):
                for j in range(NCHUNK):
                    G = NPCH + NCHUNK * p + j
                    k, pos = chunk_grp[G]
                    q = next(i for i, (a, b) in enumerate(JQ) if a <= j < b)
                    ga, gb = NSLOTS * p + 2 * q, NSLOTS * p + 2 * q + 1
                    if pos == 0:
                        # psum half reuse: group k-2's drain copies done
                        t.wait_ge(asem, acount[k - 2])
                    m0, mw = CHUNKS[j]
                    cols = mw * INNER
                    off = 512 * (j - JQ[q][0])
                    bank = ps[k % 2]
                    if j == JQ[q][0]:
                        t.wait_ge(gsem[ga % NSLOTS], 16 * (ga // NSLOTS + 1))
                    t.matmul(bank[0:B, 512 * pos:512 * pos + cols],
                             hdr_sb[:, XO + p * 2 * B:XO + p * 2 * B + B],
                             ring[ga % NSLOTS][:, off:off + cols],
                             start=True, stop=True).then_inc(pe_mm, 1)
                    if j == JQ[q][0]:
                        t.wait_ge(gsem[gb % NSLOTS], 16 * (gb // NSLOTS + 1))
                    t.matmul(bank[B:2 * B, 512 * pos:512 * pos + cols],
                             hdr_sb[:, XO + p * 2 * B + B:XO + (p + 1) * 2 * B],
                             ring[gb % NSLOTS][:, off:off + cols],
                             start=True, stop=True).then_inc(pe_mm, 1)

        # ---- ACT: PSUM -> SBUF drain copies (bf16) + output DMAs ----------
        @block.scalar
        def _(s):
            for k in range(NGRP):
                if k >= 2 and len(grp_chunks(k)) == 4:
                    # vrow slot reuse: pool mults of group k-4 done
                    if k >= 6:
                        s.wait_ge(pmul, pm_after[k - 4])
                    dst = vp if k == 0 else vrow[k % 4]
                    # two half-copies so the drain interleaves finer
                    s.wait_ge(pe_mm, mm_after[grp_chunks(k)[1]])
                    s.copy(dst[:, :1024],
                           ps[k % 2][:, :1024]).then_inc(asem, 1)
                    s.wait_ge(pe_mm, mm_after_grp[k])
                    s.copy(dst[:, 1024:2048],
                           ps[k % 2][:, 1024:2048]).then_inc(asem, 1)
                elif k == 0:
                    s.wait_ge(pe_mm, mm_after[grp_chunks(0)[1]])
                    s.copy(vp[:, :1024], ps[0][0:B, :1024]).then_inc(asem, 1)
                    s.wait_ge(pe_mm, mm_after_grp[0])
                    s.copy(vp[:, 1024:2048],
                           ps[0][0:B, 1024:2048]).then_inc(asem, 1)
                elif k == 1:
                    s.wait_ge(pe_mm, mm_after_grp[1])
                    s.copy(vp[:, 2048:], ps[1][0:B, :256]).then_inc(asem, 1)
                else:
                    p, i = divmod(k - 2, 9)
                    s.wait_ge(pe_mm, mm_after_grp[k])
                    if k >= 6:
                        s.wait_ge(pmul, pm_after[k - 4])
                    s.copy(vrow[k % 4][:, :256],
                           ps[k % 2][:, :256]).then_inc(asem, 1)
                if k == 2:
                    s.wait_ge(dself, 1)
                    s.dma_start(outp.ap()[:], accp[:]).then_inc(osem, 16)
                if k >= 11 and (k - 2) % 9 == 0:
                    # pair (k-2)//9 - 1 finished: ship it
                    pq = (k - 2) // 9 - 1
                    s.wait_ge(dself, 10 * pq + 11)
                    s.dma_start(out2.ap()[:, pq, :],
                                acc2[pq % 2][:]).then_inc(osem, 16)
            s.wait_ge(dself, 10 * (NPAIR - 1) + 11)
            s.dma_start(out2.ap()[:, NPAIR - 1, :],
                        acc2[(NPAIR - 1) % 2][:]).then_inc(osem, 16)

        # ---- Pool: attention multiplies + first-level pair sums -----------
        @block.gpsimd
        def _(g_):
            g_.wait_ge(hdrx, 16)
            attn4 = hdr_sb[:, AO:AO + NPAIR * N * HEADS].rearrange(
                "q (p m h) -> q p m h", p=NPAIR, m=N)
            attnp4 = hdr_sb[0:B, PO:PO + 9 * HEADS].rearrange(
                "q (m h) -> q m h", m=9)
            for k in range(NGRP):
                g_.wait_ge(asem, acount[k] - (1 if (k >= 2 and
                           len(grp_chunks(k)) == 4) else 0))
                if k == 0:
                    g_.tensor_tensor(
                        sclp[:, :2048].rearrange("q (m h d) -> q m h d",
                                                 m=8, h=HEADS),
                        vp[:, :2048].rearrange("q (m h d) -> q m h d",
                                               m=8, h=HEADS),
                        attnp4[:, 0:8, :, None].to_broadcast(
                            (B, 8, HEADS, DH)),
                        mybir.AluOpType.mult).then_inc(pmul, 1)
                    continue
                if k == 1:
                    g_.tensor_tensor(
                        sclp[:, 2048:].rearrange("q (m h d) -> q m h d",
                                                 m=1, h=HEADS),
                        vp[:, 2048:].rearrange("q (m h d) -> q m h d",
                                               m=1, h=HEADS),
                        attnp4[:, 8:9, :, None].to_broadcast(
                            (B, 1, HEADS, DH)),
                        mybir.AluOpType.mult).then_inc(pmul, 1)
                    continue
                p, i = divmod(k - 2, 9)
                m0 = 8 * i
                if i == 0 and p >= 2:
                    # racc/osl[p%2] reuse: DVE finished pair p-2
                    g_.wait_ge(dself, 10 * (p - 2) + 11)
                if i < 8:
                    li = 8 * p + i          # global L1-add index
                    if li >= 2:
                        g_.wait_ge(padd, li - 1)      # scr[i%2] free (DVE)
                    # two 4-key half multiplies, each right after its copy
                    for h2 in range(2):
                        if h2 == 1:
                            g_.wait_ge(asem, acount[k])
                        g_.tensor_tensor(
                            scr[i % 2][:, 1024 * h2:1024 * (h2 + 1)]
                            .rearrange("q (m h d) -> q m h d", m=4, h=HEADS),
                            vrow[k % 4][:, 1024 * h2:1024 * (h2 + 1)]
                            .rearrange("q (m h d) -> q m h d", m=4, h=HEADS),
                            attn4[:, p, m0 + 4 * h2:m0 + 4 * (h2 + 1), :,
                                  None].to_broadcast((2 * B, 4, HEADS, DH)),
                            mybir.AluOpType.mult).then_inc(pmul, 1)
                else:
                    g_.tensor_tensor(
                        osl[p % 2][:].rearrange("q (m h d) -> q m h d",
                                                m=1, h=HEADS),
                        vrow[k % 4][:, :INNER].rearrange(
                            "q (m h d) -> q m h d", m=1, h=HEADS),
                        attn4[:, p, m0:m0 + 1, :, None].to_broadcast(
                            (2 * B, 1, HEADS, DH)),
                        mybir.AluOpType.mult).then_inc(pmul, 1)


        # ---- DVE: partial reduce + bf16 fold tree per pair ----------------
        # dself incs: 1 (partial) + 8 per pair.
        @block.vector
        def _(v):
            v.wait_ge(pmul, 2)
            with nc.allow_low_precision(reason="bf16 out, tol 2e-2"):
                v.tensor_reduce(
                    accp[:].rearrange("q (h d) -> q h d", h=HEADS),
                    sclp[:].rearrange("q (m h d) -> q h d m", m=9, h=HEADS),
                    axis=mybir.AxisListType.X,
                    op=mybir.AluOpType.add).then_inc(dself, 1)
            dcnt = 1                      # dself value so far
            for p in range(NPAIR):
                pl = p % 2
                if p >= 2:
                    # all out DMAs issued so far (outp + pairs 0..p-1) done
                    v.wait_ge(osem, 16 * (p + 1))
                for i in range(8):
                    v.wait_ge(pmul, pm_after[2 + 9 * p + i])  # mults done
                    v.wait_ge(dself, dcnt)            # serialize the chain
                    if i == 0:
                        v.tensor_tensor(racc[pl][:], scr[0][:, :1024],
                                        scr[0][:, 1024:],
                                        mybir.AluOpType.add).then_inc(
                                            padd, 1)
                    else:
                        v.tensor_tensor(tb[pl][:], scr[i % 2][:, :1024],
                                        scr[i % 2][:, 1024:],
                                        mybir.AluOpType.add).then_inc(
                                            padd, 1)
                        v.wait_ge(padd, 8 * p + i + 1)
                        v.tensor_tensor(racc[pl][:], racc[pl][:], tb[pl][:],
                                        mybir.AluOpType.add).then_inc(
                                            dself, 1)
                        dcnt += 1
                v.wait_ge(dself, dcnt)
                v.tensor_tensor(ub[pl][0][:, :512], racc[pl][:, :512],
                                racc[pl][:, 512:],
                                mybir.AluOpType.add).then_inc(dself, 1)
                dcnt += 1
                v.wait_ge(dself, dcnt)
                v.tensor_tensor(ta[pl][:, :256], ub[pl][0][:, :256],
                                ub[pl][0][:, 256:512],
                                mybir.AluOpType.add).then_inc(dself, 1)
                dcnt += 1
                v.wait_ge(pmul, pm_after[2 + 9 * p + 8])  # odd mult done
                v.wait_ge(dself, dcnt)
                v.tensor_tensor(acc2[pl][:], ta[pl][:, :256], osl[pl][:],
                                mybir.AluOpType.add).then_inc(dself, 1)
                dcnt += 1
    return nc


def _to_bf16(a):
    import ml_dtypes
    return np.asarray(a, dtype=ml_dtypes.bfloat16)


def host_prep(x, gamma, beta, Wqk, Wv, Wout, bout):
    """LayerNorm + qk + softmax on host; build per-core in_maps."""
    x = np.asarray(x, np.float32)
    mu = x.mean(-1, keepdims=True)
    var = np.square(x - mu).mean(-1, keepdims=True)
    xn = ((x - mu) / np.sqrt(var + EPS) * np.asarray(gamma, np.float32)
          + np.asarray(beta, np.float32)).astype(np.float32)

    qk = xn @ np.asarray(Wqk, np.float32)
    q, k = qk[..., :INNER], qk[..., INNER:]
    q = q.reshape(B, N, HEADS, DH).transpose(0, 2, 1, 3)
    k = k.reshape(B, N, HEADS, DH).transpose(0, 2, 1, 3)
    dots = np.einsum("bhnd,bhmd->bhnm", q, k) * (DH ** -0.5)
    dots -= dots.max(-1, keepdims=True)
    e = np.exp(dots)
    attn = (e / e.sum(-1, keepdims=True)).astype(np.float32)  # [b,h,n,m]

    # [n, d, m*e] bf16 weight stream source
    WvT = np.ascontiguousarray(
        _to_bf16(Wv).transpose(0, 2, 1, 3).reshape(N, DIM, N * INNER))
    xnT = _to_bf16(xn.transpose(2, 1, 0))       # [d, n, b]

    in_maps = []
    for c in range(8):
        rows = [8 * c + i for i in range(8)]
        # wv stream: rows in natural order [A0|B0|A1|B1|...]; quarter
        # granules address strided slices of this layout directly.
        wv_g = np.empty((DIM, 2 * NPAIR * ROW_COLS), WvT.dtype)
        for p in range(NPAIR):
            wv_g[:, (2 * p) * ROW_COLS:(2 * p + 1) * ROW_COLS] = \
                WvT[rows[2 * p]]
            wv_g[:, (2 * p + 1) * ROW_COLS:(2 * p + 2) * ROW_COLS] = \
                WvT[rows[2 * p + 1]]
        # partial row m-range
        mstart, mcount = 8 * c, (9 if c == 7 else 8)
        wvp_c = np.zeros((DIM, P_COLS), WvT.dtype)
        wvp_c[:, :mcount * INNER] = WvT[64][
            :, mstart * INNER:(mstart + mcount) * INNER]
        # activations / attention
        xnT2_c = np.empty((DIM, NPAIR, 2 * B), xnT.dtype)
        attn2_c = np.empty((2 * B, NPAIR, N, HEADS), np.float32)
        for p in range(NPAIR):
            for par in range(2):
                nrow = rows[2 * p + par]
                xnT2_c[:, p, par * B:(par + 1) * B] = xnT[:, nrow, :]
                attn2_c[par * B:(par + 1) * B, p] = \
                    attn[:, :, nrow, :].transpose(0, 2, 1)
        attnp_c = np.zeros((B, 9, HEADS), np.float32)
        attnp_c[:, :mcount] = attn[
            :, :, 64, mstart:mstart + mcount].transpose(0, 2, 1)
        import ml_dtypes
        hdr_c = np.zeros((DIM, 512 + 64 + P_COLS + NPAIR * N * HEADS
                          + 9 * HEADS), ml_dtypes.bfloat16)
        hdr_c[:, 0:512] = _to_bf16(xnT2_c.reshape(DIM, 512))
        hdr_c[:, 512:576] = _to_bf16(xn[:, 64, :].T)
        hdr_c[:, 576:576 + P_COLS] = wvp_c
        ao = 576 + P_COLS
        hdr_c[:, ao:ao + NPAIR * N * HEADS] = _to_bf16(
            attn2_c.reshape(2 * B, NPAIR * N * HEADS))
        hdr_c[0:B, ao + NPAIR * N * HEADS:] = _to_bf16(
            attnp_c.reshape(B, 9 * HEADS))
        in_maps.append({"wv": wv_g, "hdr": hdr_c})
    return in_maps, xn, attn


def assemble(results, Wout, bout):
    out_pre = np.zeros((B, N, INNER), np.float32)
    for c in range(8):
        o2 = np.asarray(results[c]["out2"], np.float32)  # [2B, NPAIR, INNER]
        for p in range(NPAIR):
            out_pre[:, 8 * c + 2 * p, :] = o2[:B, p]
            out_pre[:, 8 * c + 2 * p + 1, :] = o2[B:, p]
        out_pre[:, 64, :] += np.asarray(results[c]["outp"], np.float32)
    out = out_pre.reshape(B * N, INNER) @ np.asarray(Wout, np.float32) \
        + np.asarray(bout, np.float32)
    return out.reshape(B, N, DIM).astype(np.float32)


def kernel(x, gamma, beta, Wqk, Wv, Wout, bout):
    in_maps, _, _ = host_prep(x, gamma, beta, Wqk, Wv, Wout, bout)
    if "nc" not in _CACHED:
        _CACHED["nc"] = _build_program()
    res = run_bass_kernel_spmd(_CACHED["nc"], in_maps, list(range(8))).results
    return assemble(res, Wout, bout)


# revision 51
# speedup vs baseline: 1.0569x; 1.0051x over previous
"""Trainium2 kernel for nn_Attention_50182397886533.

Computation: LayerNorm + q/k + softmax on host (<3% of FLOPs); device
computes the dominant per-(query,key) value projection
    v[b,n,m,:] = xn[b,n,:] @ Wv[n,m]            (Wv: [65,65,128,256])
fused with the attention-weighted reduction over keys m.  Host applies
the small final Wout projection.

Sharding: query rows n across 8 cores — 8 full rows per core, plus the
straggler row 64 split over m (8 keys/core, core 7 gets 9).

Per-core device pipeline (DMA-bound at ~360 GB/s, ~108us stream):
  - Wv streamed as bf16 row-piece granules through a 10-slot SBUF ring
    issued by the sync engine (SP); piece sizes shrink toward the end of
    each row so the serial drain tail stays short.
  - Rows processed in PAIRS: each matmul chunk (2 keys x 256) for row A
    goes to PSUM partitions 0:64, row B to 64:128 of the same 4-bank
    half, so all post-matmul work runs at full 128-partition width.
  - ACT (scalar) drains PSUM -> SBUF as bf16 1024-wide copies (GPSIMD
    cannot access PSUM; DVE reads of PSUM get no fast modes).
  - Pool (gpsimd) applies the attention weights: one 8-key-wide
    tensor_tensor multiply per drained group, SBUF-only.
  - DVE sums over keys with a bf16 binary fold tree (packed bf16 adds
    get the DVE 2x mode; tensor_reduce would run at 1x), interleaved
    with pool progress, then ships acc via ACT-issued output DMAs.
"""

import numpy as np

import concourse.bass as bass
import concourse.mybir as mybir
from concourse.bass_utils import run_bass_kernel_spmd

B = 64
N = 65
DIM = 128
HEADS = 8
DH = 32
INNER = 256
EPS = 1e-5

NPAIR = 4            # row pairs per core
NCHUNK = 33          # m-chunks per row (32 of width 2 + 1 of width 1)
ROW_COLS = N * INNER           # 16640 bf16 cols per row stream
NPCH = 5             # partial-row chunks (9 m slots: 4x2+1)
P_COLS = 9 * INNER             # partial-row stream cols (2304)

# chunk tables: (m0, mw) per j
CHUNKS = [(2 * j, 2) for j in range(32)] + [(64, 1)]
PCHUNKS = [(2 * j, 2) for j in range(4)] + [(8, 1)]

# Rows are streamed in QUARTERS; 8 granules per pair in consumption
# order A-q0, B-q0, A-q1, B-q1, ... Quarter q covers chunks JQ[q].
# weighted so the LAST quarter (the serial tail) is smallest
# DMA pieces per row (last ones small: they gate the serial drain tail)
JQ = [(0, 10), (10, 20), (20, 28), (28, 33)]
Q_OFF = [0, 5120, 10240, 14336]                  # col offset of piece in row
Q_LEN = [5120, 5120, 4096, 2304]                 # cols per piece
NQ = len(JQ)
NSLOTS = 2 * NQ                # ring slots: one per piece-granule of a pair


# Pool owns all multiply chunks; DVE owns the reductions.

_CACHED = {}


def _granule(g):
    """Granule g -> (dram col start, length). 2*NQ per pair: A/B per piece."""
    p, k = divmod(g, 2 * NQ)
    q, par = divmod(k, 2)
    base = p * 2 * ROW_COLS + par * ROW_COLS
    return base + Q_OFF[q], Q_LEN[q]


def _build_program():
    nc = bass.Bass()
    f32, bf16 = mybir.dt.float32, mybir.dt.bfloat16

    wv = nc.dram_tensor("wv", [DIM, 2 * NPAIR * ROW_COLS], bf16,
                        kind="ExternalInput")
    # header blob: xnT2 | xnTp | wvp | attn2 | attnp packed column-wise
    HDR_COLS = 512 + 64 + P_COLS + NPAIR * N * HEADS + 9 * HEADS
    XO, XPO = 0, 512
    WO = 576
    AO = WO + P_COLS
    PO = AO + NPAIR * N * HEADS
    hdr = nc.dram_tensor("hdr", [DIM, HDR_COLS], bf16, kind="ExternalInput")
    out2 = nc.dram_tensor("out2", [2 * B, NPAIR, INNER], bf16,
                          kind="ExternalOutput")
    outp = nc.dram_tensor("outp", [B, INNER], bf16, kind="ExternalOutput")

    # build-time schedules -------------------------------------------------
    # global chunk order: partial-row chunks first (G 0..4), then pairs
    # (G = NPCH + NCHUNK*p + j).  Pool owns every multiply in this order.
    NG = NPCH + NPAIR * NCHUNK                # 137
    # matmul count after chunk G fully issued (pair chunks have 2 matmuls)
    mm_after = {}
    mm = 0
    for G in range(NG):
        mm += 1 if G < NPCH else 2
        mm_after[G] = mm
    # 4-chunk groups: partial row = groups 0,1; pair p group i (i<9) is
    # global group 2 + 9p + i (8 full groups of 4 chunks + chunk 32 alone).
    NGRP = 2 + 9 * NPAIR
    def grp_chunks(k):
        if k == 0:
            return list(range(4))
        if k == 1:
            return [4]
        p, i = divmod(k - 2, 9)
        j0 = 4 * i
        return [NPCH + NCHUNK * p + j for j in range(j0, min(j0 + 4, NCHUNK))]
    mm_after_grp = [mm_after[grp_chunks(k)[-1]] for k in range(NGRP)]
    acount = []
    pm_after = []
    _a = _m = 0
    for k in range(NGRP):
        full = len(grp_chunks(k)) == 4
        _a += 2 if full else 1
        acount.append(_a)
        _m += 2 if (full and k >= 2) else 1
        pm_after.append(_m)

    import contextlib
    with contextlib.ExitStack() as st:
        ring = [st.enter_context(nc.sbuf_tensor(f"ring{s}",
                                                [DIM, Q_LEN[s // 2]], bf16))
                for s in range(NSLOTS)]
        hdr_sb = st.enter_context(nc.sbuf_tensor([DIM, HDR_COLS], bf16))
        # PSUM: two 4-bank halves; groups alternate between them
        psA = st.enter_context(nc.psum_tensor([2 * B, 2048], f32))
        psB = st.enter_context(nc.psum_tensor([2 * B, 2048], f32))
        # ACT drain targets: 4-slot ring of raw v (bf16), partial-row vp
        vrow = [st.enter_context(nc.sbuf_tensor(f"vrow{s}", [2 * B, 2048],
                                                bf16)) for s in range(4)]
        vp = st.enter_context(nc.sbuf_tensor([B, P_COLS], bf16))
        # pool mult outputs: 2-slot scratch; L1 pair-sums; odd m=64 slice
        scr = [st.enter_context(nc.sbuf_tensor(f"scr{s}", [2 * B, 2048],
                                               bf16)) for s in range(2)]
        racc = [st.enter_context(nc.sbuf_tensor(f"racc{s}", [2 * B, 1024],
                                                bf16)) for s in range(2)]
        tb = [st.enter_context(nc.sbuf_tensor(f"tb{s}", [2 * B, 1024], bf16))
              for s in range(2)]
        osl = [st.enter_context(nc.sbuf_tensor(f"osl{s}", [2 * B, INNER],
                                               bf16)) for s in range(2)]
        sclp = st.enter_context(nc.sbuf_tensor([B, P_COLS], bf16))
        # DVE fold temps
        ta = [st.enter_context(nc.sbuf_tensor(f"ta{s}", [2 * B, 2048], bf16))
              for s in range(2)]
        ub = [[st.enter_context(nc.sbuf_tensor(f"ub{s}_{q}", [2 * B, 1024],
                                               bf16)) for q in range(2)]
              for s in range(2)]
        acc2 = [st.enter_context(nc.sbuf_tensor(f"acc2{s}", [2 * B, INNER],
                                                bf16)) for s in range(2)]
        accp = st.enter_context(nc.sbuf_tensor([B, INNER], bf16))

        hdrx = st.enter_context(nc.semaphore("hdrx"))
        hdra = st.enter_context(nc.semaphore("hdra"))
        gsem = [st.enter_context(nc.semaphore(f"gsem{s}"))
                for s in range(NSLOTS)]
        gpsem = st.enter_context(nc.semaphore("gpsem"))
        pe_mm = st.enter_context(nc.semaphore("pe_mm"))
        asem = st.enter_context(nc.semaphore("asem"))
        pmul = st.enter_context(nc.semaphore("pmul"))
        padd = st.enter_context(nc.semaphore("padd"))
        dself = st.enter_context(nc.semaphore("dself"))
        osem = st.enter_context(nc.semaphore("osem"))
        block = st.enter_context(nc.Block())

        # chunk -> (group, position) map
        chunk_grp = {}
        for k in range(NGRP):
            for pos, G in enumerate(grp_chunks(k)):
                chunk_grp[G] = (k, pos)
        ps = [psA, psB]

        # ---- sync engine: input DMA stream --------------------------------
        @block.sync
        def _(sy):
            sy.dma_start(hdr_sb[:], hdr.ap()).then_inc(hdrx, 16)
            for g in range(2 * NQ * NPAIR):
                if g >= NSLOTS:
                    # slot reuse: matmuls consuming the same piece of the
                    # previous pair must be done
                    gp, kp = divmod(g - NSLOTS, NSLOTS)
                    jlast = JQ[kp // 2][1] - 1
                    sy.wait_ge(pe_mm, mm_after[NPCH + NCHUNK * gp + jlast])
                c0, clen = _granule(g)
                sy.dma_start(ring[g % NSLOTS][:, :clen],
                             wv.ap()[:, c0:c0 + clen]).then_inc(
                                 gsem[g % NSLOTS], 16)

        # ---- tensor engine: value-projection matmuls ----------------------
        @block.tensor
        def _(t):
            t.wait_ge(hdrx, 16)         # header blob loaded
            for j5 in range(NPCH):
                k, pos = chunk_grp[j5]
                m0, mw = PCHUNKS[j5]
                cols = mw * INNER
                t.matmul(ps[k % 2][0:B, 512 * pos:512 * pos + cols],
                         hdr_sb[:, XPO:XPO + B],
                         hdr_sb[:, WO + 512 * j5:WO + 512 * j5 + cols],
                         start=True, stop=True).then_inc(pe_mm, 1)
            for p in range(NPAIR):
                for j in range(NCHUNK):
                    G = NPCH + NCHUNK * p + j
                    k, pos = chunk_grp[G]
                    q = next(i for i, (a, b) in enumerate(JQ) if a <= j < b)
                    ga, gb = NSLOTS * p + 2 * q, NSLOTS * p + 2 * q + 1
                    if pos == 0:
                        # psum half reuse: group k-2's drain copies done
                        t.wait_ge(asem, acount[k - 2])
                    m0, mw = CHUNKS[j]
                    cols = mw * INNER
                    off = 512 * (j - JQ[q][0])
                    bank = ps[k % 2]
                    if j == JQ[q][0]:
                        t.wait_ge(gsem[ga % NSLOTS], 16 * (ga // NSLOTS + 1))
                    t.matmul(bank[0:B, 512 * pos:512 * pos + cols],
                             hdr_sb[:, XO + p * 2 * B:XO + p * 2 * B + B],
                             ring[ga % NSLOTS][:, off:off + cols],
                             start=True, stop=True).then_inc(pe_mm, 1)
                    if j == JQ[q][0]:
                        t.wait_ge(gsem[gb % NSLOTS], 16 * (gb // NSLOTS + 1))
                    t.matmul(bank[B:2 * B, 512 * pos:512 * pos + cols],
                             hdr_sb[:, XO + p * 2 * B + B:XO + (p + 1) * 2 * B],
                             ring[gb % NSLOTS][:, off:off + cols],
                             start=True, stop=True).then_inc(pe_mm, 1)

        # ---- ACT: PSUM -> SBUF drain copies (bf16) + output DMAs ----------
        @block.scalar
        def _(s):
            for k in range(NGRP):
                if k >= 2 and len(grp_chunks(k)) == 4:
                    # vrow slot reuse: pool mults of group k-4 done
                    if k >= 6:
                        s.wait_ge(pmul, pm_after[k - 4])
                    dst = vp if k == 0 else vrow[k % 4]
                    # two half-copies so the drain interleaves finer
                    s.wait_ge(pe_mm, mm_after[grp_chunks(k)[1]])
                    s.copy(dst[:, :1024],
                           ps[k % 2][:, :1024]).then_inc(asem, 1)
                    s.wait_ge(pe_mm, mm_after_grp[k])
                    s.copy(dst[:, 1024:2048],
                           ps[k % 2][:, 1024:2048]).then_inc(asem, 1)
                elif k == 0:
                    s.wait_ge(pe_mm, mm_after[grp_chunks(0)[1]])
                    s.copy(vp[:, :1024], ps[0][0:B, :1024]).then_inc(asem, 1)
                    s.wait_ge(pe_mm, mm_after_grp[0])
                    s.copy(vp[:, 1024:2048],
                           ps[0][0:B, 1024:2048]).then_inc(asem, 1)
                elif k == 1:
                    s.wait_ge(pe_mm, mm_after_grp[1])
                    s.copy(vp[:, 2048:], ps[1][0:B, :256]).then_inc(asem, 1)
                else:
                    p, i = divmod(k - 2, 9)
                    s.wait_ge(pe_mm, mm_after_grp[k])
                    if k >= 6:
                        s.wait_ge(pmul, pm_after[k - 4])
                    s.copy(vrow[k % 4][:, :256],
                           ps[k % 2][:, :256]).then_inc(asem, 1)
                if k == 2:
                    s.wait_ge(dself, 1)
                    s.dma_start(outp.ap()[:], accp[:]).then_inc(osem, 16)
                if k >= 11 and (k - 2) % 9 == 0:
                    # pair (k-2)//9 - 1 finished: ship it
                    pq = (k - 2) // 9 - 1
                    s.wait_ge(dself, 18 * pq + 19)
                    s.dma_start(out2.ap()[:, pq, :],
                                acc2[pq % 2][:]).then_inc(osem, 16)
            s.wait_ge(dself, 18 * (NPAIR - 1) + 19)
            s.dma_start(out2.ap()[:, NPAIR - 1, :],
                        acc2[(NPAIR - 1) % 2][:]).then_inc(osem, 16)

        # ---- Pool: attention multiplies + first-level pair sums -----------
        @block.gpsimd
        def _(g_):
            g_.wait_ge(hdrx, 16)
            attn4 = hdr_sb[:, AO:AO + NPAIR * N * HEADS].rearrange(
                "q (p m h) -> q p m h", p=NPAIR, m=N)
            attnp4 = hdr_sb[0:B, PO:PO + 9 * HEADS].rearrange(
                "q (m h) -> q m h", m=9)
            for k in range(NGRP):
                g_.wait_ge(asem, acount[k] - (1 if (k >= 2 and
                           len(grp_chunks(k)) == 4) else 0))
                if k == 0:
                    g_.tensor_tensor(
                        sclp[:, :2048].rearrange("q (m h d) -> q m h d",
                                                 m=8, h=HEADS),
                        vp[:, :2048].rearrange("q (m h d) -> q m h d",
                                               m=8, h=HEADS),
                        attnp4[:, 0:8, :, None].to_broadcast(
                            (B, 8, HEADS, DH)),
                        mybir.AluOpType.mult).then_inc(pmul, 1)
                    continue
                if k == 1:
                    g_.tensor_tensor(
                        sclp[:, 2048:].rearrange("q (m h d) -> q m h d",
                                                 m=1, h=HEADS),
                        vp[:, 2048:].rearrange("q (m h d) -> q m h d",
                                               m=1, h=HEADS),
                        attnp4[:, 8:9, :, None].to_broadcast(
                            (B, 1, HEADS, DH)),
                        mybir.AluOpType.mult).then_inc(pmul, 1)
                    continue
                p, i = divmod(k - 2, 9)
                m0 = 8 * i
                if i == 0 and p >= 2:
                    # racc/osl[p%2] reuse: DVE finished pair p-2
                    g_.wait_ge(dself, 18 * (p - 2) + 19)
                if i < 8:
                    li = 8 * p + i          # global scr-use index
                    if li >= 2:
                        # scr[i%2] free: previous user's b-fold done
                        # (init at 2+18pp; b-fold j at 2+18pp+2j)
                        pp, jp = (p, i - 2) if i >= 2 else (p - 1, i + 6)
                        g_.wait_ge(dself, 2 + 18 * pp + 2 * jp)
                    # two 4-key half multiplies, each right after its copy
                    for h2 in range(2):
                        if h2 == 1:
                            g_.wait_ge(asem, acount[k])
                        g_.tensor_tensor(
                            scr[i % 2][:, 1024 * h2:1024 * (h2 + 1)]
                            .rearrange("q (m h d) -> q m h d", m=4, h=HEADS),
                            vrow[k % 4][:, 1024 * h2:1024 * (h2 + 1)]
                            .rearrange("q (m h d) -> q m h d", m=4, h=HEADS),
                            attn4[:, p, m0 + 4 * h2:m0 + 4 * (h2 + 1), :,
                                  None].to_broadcast((2 * B, 4, HEADS, DH)),
                            mybir.AluOpType.mult).then_inc(pmul, 1)
                else:
                    g_.tensor_tensor(
                        osl[p % 2][:].rearrange("q (m h d) -> q m h d",
                                                m=1, h=HEADS),
                        vrow[k % 4][:, :INNER].rearrange(
                            "q (m h d) -> q m h d", m=1, h=HEADS),
                        attn4[:, p, m0:m0 + 1, :, None].to_broadcast(
                            (2 * B, 1, HEADS, DH)),
                        mybir.AluOpType.mult).then_inc(pmul, 1)


        # ---- DVE: partial reduce + bf16 fold tree per pair ----------------
        # dself incs: 1 (partial) + 8 per pair.
        @block.vector
        def _(v):
            v.wait_ge(pmul, 2)
            with nc.allow_low_precision(reason="bf16 out, tol 2e-2"):
                v.tensor_reduce(
                    accp[:].rearrange("q (h d) -> q h d", h=HEADS),
                    sclp[:].rearrange("q (m h d) -> q h d m", m=9, h=HEADS),
                    axis=mybir.AxisListType.X,
                    op=mybir.AluOpType.add).then_inc(dself, 1)
            # Direct scr->racc half-folds; every op increments dself so
            # the whole chain lives on one semaphore (detector-friendly).
            # Per pair: init + 7x2 folds + d,e,f = 18 incs; f(p) = 18p+19.
            dcnt = 1
            for p in range(NPAIR):
                pl = p % 2
                if p >= 2:
                    # all out DMAs issued so far (outp + pairs 0..p-1) done
                    v.wait_ge(osem, 16 * (p + 1))
                for i in range(8):
                    k = 2 + 9 * p + i
                    if i == 0:
                        v.wait_ge(pmul, pm_after[k])
                        v.wait_ge(dself, dcnt)
                        v.tensor_tensor(racc[pl][:], scr[0][:, :1024],
                                        scr[0][:, 1024:],
                                        mybir.AluOpType.add).then_inc(
                                            dself, 1)
                        dcnt += 1
                    else:
                        v.wait_ge(pmul, pm_after[k] - 1)
                        v.wait_ge(dself, dcnt)
                        v.tensor_tensor(racc[pl][:], racc[pl][:],
                                        scr[i % 2][:, :1024],
                                        mybir.AluOpType.add).then_inc(
                                            dself, 1)
                        dcnt += 1
                        v.wait_ge(pmul, pm_after[k])
                        v.wait_ge(dself, dcnt)
                        v.tensor_tensor(racc[pl][:], racc[pl][:],
                                        scr[i % 2][:, 1024:],
                                        mybir.AluOpType.add).then_inc(
                                            dself, 1)
                        dcnt += 1
                v.wait_ge(dself, dcnt)
                v.tensor_tensor(ub[pl][0][:, :512], racc[pl][:, :512],
                                racc[pl][:, 512:],
                                mybir.AluOpType.add).then_inc(dself, 1)
                dcnt += 1
                v.wait_ge(dself, dcnt)
                v.tensor_tensor(ta[pl][:, :256], ub[pl][0][:, :256],
                                ub[pl][0][:, 256:512],
                                mybir.AluOpType.add).then_inc(dself, 1)
                dcnt += 1
                v.wait_ge(pmul, pm_after[2 + 9 * p + 8])  # odd mult done
                v.wait_ge(dself, dcnt)
                v.tensor_tensor(acc2[pl][:], ta[pl][:, :256], osl[pl][:],
                                mybir.AluOpType.add).then_inc(dself, 1)
                dcnt += 1
    return nc


def _to_bf16(a):
    import ml_dtypes
    return np.asarray(a, dtype=ml_dtypes.bfloat16)


def host_prep(x, gamma, beta, Wqk, Wv, Wout, bout):
    """LayerNorm + qk + softmax on host; build per-core in_maps."""
    x = np.asarray(x, np.float32)
    mu = x.mean(-1, keepdims=True)
    var = np.square(x - mu).mean(-1, keepdims=True)
    xn = ((x - mu) / np.sqrt(var + EPS) * np.asarray(gamma, np.float32)
          + np.asarray(beta, np.float32)).astype(np.float32)

    qk = xn @ np.asarray(Wqk, np.float32)
    q, k = qk[..., :INNER], qk[..., INNER:]
    q = q.reshape(B, N, HEADS, DH).transpose(0, 2, 1, 3)
    k = k.reshape(B, N, HEADS, DH).transpose(0, 2, 1, 3)
    dots = np.einsum("bhnd,bhmd->bhnm", q, k) * (DH ** -0.5)
    dots -= dots.max(-1, keepdims=True)
    e = np.exp(dots)
    attn = (e / e.sum(-1, keepdims=True)).astype(np.float32)  # [b,h,n,m]

    # [n, d, m*e] bf16 weight stream source
    WvT = np.ascontiguousarray(
        _to_bf16(Wv).transpose(0, 2, 1, 3).reshape(N, DIM, N * INNER))
    xnT = _to_bf16(xn.transpose(2, 1, 0))       # [d, n, b]

    in_maps = []
    for c in range(8):
        rows = [8 * c + i for i in range(8)]
        # wv stream: rows in natural order [A0|B0|A1|B1|...]; quarter
        # granules address strided slices of this layout directly.
        wv_g = np.empty((DIM, 2 * NPAIR * ROW_COLS), WvT.dtype)
        for p in range(NPAIR):
            wv_g[:, (2 * p) * ROW_COLS:(2 * p + 1) * ROW_COLS] = \
                WvT[rows[2 * p]]
            wv_g[:, (2 * p + 1) * ROW_COLS:(2 * p + 2) * ROW_COLS] = \
                WvT[rows[2 * p + 1]]
        # partial row m-range
        mstart, mcount = 8 * c, (9 if c == 7 else 8)
        wvp_c = np.zeros((DIM, P_COLS), WvT.dtype)
        wvp_c[:, :mcount * INNER] = WvT[64][
            :, mstart * INNER:(mstart + mcount) * INNER]
        # activations / attention
        xnT2_c = np.empty((DIM, NPAIR, 2 * B), xnT.dtype)
        attn2_c = np.empty((2 * B, NPAIR, N, HEADS), np.float32)
        for p in range(NPAIR):
            for par in range(2):
                nrow = rows[2 * p + par]
                xnT2_c[:, p, par * B:(par + 1) * B] = xnT[:, nrow, :]
                attn2_c[par * B:(par + 1) * B, p] = \
                    attn[:, :, nrow, :].transpose(0, 2, 1)
        attnp_c = np.zeros((B, 9, HEADS), np.float32)
        attnp_c[:, :mcount] = attn[
            :, :, 64, mstart:mstart + mcount].transpose(0, 2, 1)
        import ml_dtypes
        hdr_c = np.zeros((DIM, 512 + 64 + P_COLS + NPAIR * N * HEADS
                          + 9 * HEADS), ml_dtypes.bfloat16)
        hdr_c[:, 0:512] = _to_bf16(xnT2_c.reshape(DIM, 512))
        hdr_c[:, 512:576] = _to_bf16(xn[:, 64, :].T)
        hdr_c[:, 576:576 + P_COLS] = wvp_c
        ao = 576 + P_COLS
        hdr_c[:, ao:ao + NPAIR * N * HEADS] = _to_bf16(
            attn2_c.reshape(2 * B, NPAIR * N * HEADS))
        hdr_c[0:B, ao + NPAIR * N * HEADS:] = _to_bf16(
            attnp_c.reshape(B, 9 * HEADS))
        in_maps.append({"wv": wv_g, "hdr": hdr_c})
    return in_maps, xn, attn


def assemble(results, Wout, bout):
    out_pre = np.zeros((B, N, INNER), np.float32)
    for c in range(8):
        o2 = np.asarray(results[c]["out2"], np.float32)  # [2B, NPAIR, INNER]
        for p in range(NPAIR):
            out_pre[:, 8 * c + 2 * p, :] = o2[:B, p]
            out_pre[:, 8 * c + 2 * p + 1, :] = o2[B:, p]
        out_pre[:, 64, :] += np.asarray(results[c]["outp"], np.float32)
    out = out_pre.reshape(B * N, INNER) @ np.asarray(Wout, np.float32) \
        + np.asarray(bout, np.float32)
    return out.reshape(B, N, DIM).astype(np.float32)


def kernel(x, gamma, beta, Wqk, Wv, Wout, bout):
    in_maps, _, _ = host_prep(x, gamma, beta, Wqk, Wv, Wout, bout)
    if "nc" not in _CACHED:
        _CACHED["nc"] = _build_program()
    res = run_bass_kernel_spmd(_CACHED["nc"], in_maps, list(range(8))).results
    return assemble(res, Wout, bout)
